# revision 30
# baseline (speedup 1.0000x reference)
"""Recurrent linear-attention transformer on 8 Trainium2 NeuronCores.

Sharding: 4-way data parallel over batch x 2-way sequence split (halves of
S=2048). Causal linear attention is computed in chunked form; the only
cross-core dependency is the cumulative (k^T v, sum k) state at the half
boundary, exchanged once per layer via a pairwise AllReduce.

Schedule (v2, PE-density focused):
  - The input projection is folded into the causal conv on the host
    (V_d = ccW[:,:,d] @ in_W), so the conv contracts over CIN=64 only:
    3 matmuls per seq tile instead of 15.
  - LN1 for layer 0 is interleaved with the conv tiles.
  - Per layer: pass1 (v then k projections + per-chunk prefix states) ->
    boundary AllReduce -> pass2a (q projection + masked intra scores,
    overlapping the collective) -> Peff = P_local + R adds on DVE ->
    a chunk-granular pipeline over pass2b + FFN: each chunk's intra
    numerator/denominator matmuls SEED the PSUM accumulation groups
    before R arrives, the prefix matmuls accumulate on top (no identity
    re-injection), and o-projection / LN2 sweep / FFN c1 (N=256 per
    chunk) / c2 are interleaved so the attn-transpose and LN chains hide
    under PE work across the layer boundary into the next pass1.
  - Weight DMAs for layer i+1 are emitted at the top of pass2a(i): DMA
    queues are round-robin in emission order with head-of-line blocking,
    so late-emitted weight loads would queue behind dependency-stalled
    transposes.
  - LN scales use sqrt(var+eps) on ACT + reciprocal on DVE; normalize
    runs on ACT (Identity with per-partition scale/bias) to keep the
    DVE queue shallow. PSUM: tag "one" x3, "big2" (1024) x2, "sm" x1
    = 8 banks; one start per PSUM zero-region per accumulation group.
    All projection matmuls (k/q/v) draw PSUM from the deep "one" pool;
    the 2-buffer "big2" rotation serves only the attention-numerator and
    FFN-c1 groups — PSUM-drain slack is the dominant hardware lever that
    the cost-model simulator does not see (bufs=2 -> 3 on the drain tiles
    measured ~45us, this split another ~25us).

Per-core layout conventions (SBUF 2-D tensors, 128 partitions):
  lat   f32  [128, 8*512]    seq-major residual: c-tile ct8 block, cols = e
  hT    bf16 [128, 4*4*256]  feature-major LN1 output: (chunk, kt) blocks
  kT    bf16 [128, 4*4*256]  phi(k) feature-major: (chunk, ft) blocks, cols=t
  v     bf16 [128, 4*2*512]  v seq-major: (chunk, tt) blocks, cols = e'
  P_all bf16 [128, 5*4*513]  prefix states: (j, kt) -> [KV[e,e'] | s_k[e]]
All matmul operands bf16, PSUM/stats/residual f32.

`build_program(repeat=R)` unrolls the whole computation R times (identical
output each iteration); test.py uses it to measure per-iteration device time
through the ~80 ms fixed launch overhead of the axon-tunneled PJRT path.
"""

import threading

import numpy as np
import ml_dtypes

import concourse.bass as bass
import concourse.bacc as bacc
import concourse.tile as tile
import concourse.mybir as mybir
from concourse.bass_utils import run_bass_kernel_spmd

AF = mybir.ActivationFunctionType
ALU = mybir.AluOpType
F32 = mybir.dt.float32
BF16 = mybir.dt.bfloat16
BF = ml_dtypes.bfloat16

L, B, CIN, COUT, E, S = 4, 4, 64, 64, 512, 2048
NCORES = 8
SH = S // 2          # per-core sequence half
C = 256              # attention chunk
NCH = SH // C        # 4 chunks
CT = C // 128        # 2 c-tiles per chunk
KT = E // 128        # 4 feature tiles
NT8 = SH // 128      # 8 seq tiles per half
EPS = 1e-6
LN_EPS = 1e-5
SKW = E + 1          # 513: KV block plus s_k column

REPLICA_GROUPS = [[0, 1], [2, 3], [4, 5], [6, 7]]

PHASE_HOOK = None  # optional (name, next_instr_id) recorder for profiling


def _ph(nc, name):
    if PHASE_HOOK is not None:
        PHASE_HOOK(name, nc.next_id())

# brow layout (bf16): per-layer [bv, bo, c2b] rows, then 3 conv-bias rows
# (W_d @ in_b), cc_b, out_b
BROW_N = L * 3 * 512 + 3 * 512 + 512 + COUT


def build_program(cc=True, repeat=1, with_bias=False, loop_input=False, no_r=False):
    nc = bacc.Bacc("TRN2", target_bir_lowering=False, debug=False,
                   num_devices=NCORES)

    rept_d = (nc.dram_tensor("rept", [1, 1], mybir.dt.int32,
                             kind="ExternalInput") if loop_input else None)
    x_d = nc.dram_tensor("x_sl", [CIN, SH + 2], BF16, kind="ExternalInput")
    cvWT_d = nc.dram_tensor("cvWT", [CIN, 3 * E], BF16, kind="ExternalInput")
    wpack_d = nc.dram_tensor("wpack", [128, L * 6 * KT * 512], BF16, kind="ExternalInput")
    outWT_d = nc.dram_tensor("outWT", [128, KT * COUT], BF16, kind="ExternalInput")
    brow_d = nc.dram_tensor("brow", [1, BROW_N], BF16, kind="ExternalInput")
    bcol_d = nc.dram_tensor("bcol", [128, L * 3 * KT], F32, kind="ExternalInput")
    tril_d = nc.dram_tensor("tril", [128, 128], BF16, kind="ExternalInput")
    mcol_d = nc.dram_tensor("mcol", [128, 2], F32, kind="ExternalInput")  # [m, 1-m]
    halo_d = nc.dram_tensor("halo", [1, SH + 2], BF16, kind="ExternalInput")
    ones_row_d = nc.dram_tensor("ones_row", [1, 512], BF16, kind="ExternalInput")
    ones_col_bf_d = nc.dram_tensor("ones_col_bf", [128, 1], BF16, kind="ExternalInput")

    out_d = nc.dram_tensor("out", [COUT, SH], F32, kind="ExternalOutput")

    with tile.TileContext(nc, num_cores=NCORES) as tc:
        _emit(nc, tc, x_d, cvWT_d, wpack_d, outWT_d, brow_d, bcol_d,
              tril_d, mcol_d, halo_d, ones_row_d,
              ones_col_bf_d, out_d, cc=cc, repeat=repeat,
              with_bias=with_bias, rept_d=rept_d, no_r=no_r)
    nc.compile()
    return nc


def _emit(nc, tc, x_d, cvWT_d, wpack_d, outWT_d, brow_d, bcol_d,
          tril_d, mcol_d, halo_d, ones_row_d,
          ones_col_bf_d, out_d, cc=True, repeat=1,
          with_bias=False, rept_d=None, no_r=False):
    import contextlib
    ctx = contextlib.ExitStack()
    with ctx:
        singles = ctx.enter_context(tc.tile_pool(name="singles", bufs=1))
        persist = ctx.enter_context(tc.tile_pool(name="persist", bufs=1))
        small = ctx.enter_context(tc.tile_pool(name="small", bufs=4))
        psum = ctx.enter_context(tc.tile_pool(name="psum", bufs=2, space="PSUM"))
        dram = ctx.enter_context(tc.tile_pool(name="dram", bufs=2, space="DRAM"))

        dma = nc.sync.dma_start

        # ---- constants ----
        outWT = singles.tile([128, KT * COUT], BF16)
        dma(out=outWT, in_=outWT_d[:, :])
        cvWT = singles.tile([CIN, 3 * E], BF16)
        dma(out=cvWT, in_=cvWT_d[:, :])
        bcol = singles.tile([128, L * 3 * KT], F32)
        dma(out=bcol, in_=bcol_d[:, :])
        tril = singles.tile([128, 128], BF16)
        dma(out=tril, in_=tril_d[:, :])
        mcol = singles.tile([128, 2], F32)
        dma(out=mcol, in_=mcol_d[:, :])
        ones_col_bf = singles.tile([128, 1], BF16)
        dma(out=ones_col_bf, in_=ones_col_bf_d[:, :])
        eps_col = singles.tile([128, 1], F32)
        nc.vector.memset(eps_col, LN_EPS)
        if with_bias:
            brow = singles.tile([1, BROW_N], BF16)
            dma(out=brow, in_=brow_d[:, :])
            halo = singles.tile([1, SH + 2], BF16)
            dma(out=halo, in_=halo_d[:, :])
            ones_row = singles.tile([1, 512], BF16)
            dma(out=ones_row, in_=ones_row_d[:, :])

        cvb_row = lambda d: brow[:, L * 3 * 512 + d * 512: L * 3 * 512 + (d + 1) * 512]
        ccb_row = lambda: brow[:, L * 3 * 512 + 3 * 512: L * 3 * 512 + 4 * 512]
        outb_row = lambda: brow[:, L * 3 * 512 + 4 * 512: L * 3 * 512 + 4 * 512 + COUT]

        def brow_w(i, w):
            # w: 0=bv 1=bo 2=c2b
            return brow[:, (i * 3 + w) * 512:(i * 3 + w) * 512 + 512]

        def bcol_w(i, which, kt):
            # which: 0=bq 1=bk 2=c1b
            c = (i * 3 + which) * KT + kt
            return bcol[:, c:c + 1]

        # ---- persistent state ----
        lat = persist.tile([128, NT8 * 512], F32)
        P_all = persist.tile([128, (NCH + 1) * KT * SKW], BF16)
        nc.vector.memset(P_all[:, 0:KT * SKW], 0.0)
        # feature-tile-major activations: [within-tile row, tile, seq col]
        hT_all = persist.tile([128, KT, SH // 4 * 4], BF16)   # [e%128, kt, s]
        kT_all = persist.tile([128, KT, SH // 4 * 4], BF16)   # [feat%128, ft, t]
        h2T_all = persist.tile([128, KT, SH // 4 * 4], BF16)
        v_all = persist.tile([128, NCH * CT * 512], BF16)
        R_sb = persist.tile([128, KT * SKW], BF16)

        def Pb(j, kt):
            o = (j * KT + kt) * SKW
            return P_all[:, o:o + SKW]

        # layer-loop pools (persist across repeats)
        wbufs = 1 if with_bias else 2
        wq_pool = ctx.enter_context(tc.tile_pool(name="wq_pool", bufs=wbufs))
        wo_pool = ctx.enter_context(tc.tile_pool(name="wo_pool", bufs=wbufs))
        work = ctx.enter_context(tc.tile_pool(name="work", bufs=2))

        def load_weights(i):
            """DMA layer-i weights; returns (wq, wo) tiles."""
            wq = wq_pool.tile([128, 3 * KT * 512], BF16, tag="wq")
            wo = wo_pool.tile([128, 3 * KT * 512], BF16, tag="wo")
            for wti in range(3):
                dma(out=wq[:, wti * KT * 512:(wti + 1) * KT * 512],
                    in_=wpack_d[:, (i * 6 + wti) * KT * 512:(i * 6 + wti + 1) * KT * 512])
                dma(out=wo[:, wti * KT * 512:(wti + 1) * KT * 512],
                    in_=wpack_d[:, (i * 6 + 3 + wti) * KT * 512:(i * 6 + 4 + wti) * KT * 512])
            return wq, wo

        # ---------- layernorm helpers ----------
        def ln_stats(mv8, ct8, sl):
            """bn stats of lat c-tile ct8 into mv8[:, 2*sl:2*sl+2]."""
            stats = small.tile([128, 6], F32, tag="lnst")
            nc.vector.bn_stats(out=stats, in_=lat[:, ct8 * 512:(ct8 + 1) * 512])
            nc.vector.bn_aggr(out=mv8[:, 2 * sl:2 * sl + 2], in_=stats)

        def ln_scales(mv8, n, tag):
            """From interleaved [mean,var] pairs build rstd [128,n] and
            nb = -mean*rstd [128,n] (ACT sqrt + DVE reciprocal)."""
            sd = small.tile([128, n], F32, tag=tag + "sd")
            rstd = small.tile([128, n], F32, tag=tag + "rs")
            nb = small.tile([128, n], F32, tag=tag + "nb")
            mvv = mv8[:, 0:2 * n].rearrange("p (n two) -> p n two", two=2)
            nc.scalar.activation(out=sd, in_=mvv[:, :, 1], func=AF.Sqrt,
                                 bias=eps_col[:, 0:1], scale=1.0)
            nc.vector.reciprocal(out=rstd, in_=sd)
            nc.vector.scalar_tensor_tensor(out=nb, in0=mvv[:, :, 0], scalar=-1.0,
                                           in1=rstd, op0=ALU.mult, op1=ALU.mult)
            return nb, rstd

        def ln_norm(dst_y, ct8, nb, rstd, sl):
            nc.scalar.activation(out=dst_y, in_=lat[:, ct8 * 512:(ct8 + 1) * 512],
                                 func=AF.Identity, bias=nb[:, sl:sl + 1],
                                 scale=rstd[:, sl:sl + 1])

        def ln1_sweep(mv, jp):
            """Per-pair LN1 finish: rsqrt batch, normalize, transpose to hT."""
            nb, rstd = ln_scales(mv[:, jp * CT * 2: jp * CT * 2 + 8], 4, "l1")
            for c4 in range(4):
                y = work.tile([128, 512], BF16, tag="y", bufs=2)
                ln_norm(y, jp * CT + c4, nb, rstd, c4)
                nc.sync.dma_start_transpose(
                    out=hT_all[:, :, jp * 256 + c4 * 128: jp * 256 + c4 * 128 + 128],
                    in_=y)

        def pair_proj_phi(wt, w, jp, dst3, i, which):
            """Feature-major projection for a chunk pair with phi applied.
            dst3: 3D [128, KT, SH] tile written at cols [jp*256, jp*256+512).
            PSUM comes from the deep 'one' pool so the big2 rotation stays
            exclusive to the attention-numerator / FFN-c1 pipeline."""
            for fh in range(2):
                pps = []
                for fi in range(2):
                    pp = psum.tile([128, 512], F32, tag="one", bufs=3)
                    pps.append(pp)
                    ft = fh * 2 + fi
                    for kt in range(KT):
                        nc.tensor.matmul(
                            pp[:, :],
                            wt[:, (w * KT + kt) * 512 + ft * 128:(w * KT + kt) * 512 + ft * 128 + 128],
                            hT_all[:, kt, jp * 256: jp * 256 + 512],
                            start=(kt == 0), stop=(kt == KT - 1))
                et = work.tile([128, 1024], BF16, tag="phiE")
                for fi in range(2):
                    ft = fh * 2 + fi
                    bc = bcol_w(i, which, ft)
                    nc.scalar.activation(out=et[:, fi * 512:(fi + 1) * 512],
                                         in_=pps[fi][:, :],
                                         func=AF.Exp, bias=bc, scale=1.0)
                    nc.scalar.activation(out=dst3[:, ft, jp * 256: jp * 256 + 512],
                                         in_=pps[fi][:, :],
                                         func=AF.Relu, bias=bc, scale=1.0)
                d = dst3[:, fh * 2: fh * 2 + 2, jp * 256: jp * 256 + 512]
                nc.vector.scalar_tensor_tensor(out=d, in0=et, scalar=1.0, in1=d,
                                               op0=ALU.min, op1=ALU.add)

        def pass1_pair(i, jp, wq):
            """k/v projections + prefix-state chain for chunks jp, jp+1.
            v matmuls are emitted before the k projection so their PSUM
            drains are not queued behind the phi activations."""
            for j in (jp, jp + 1):
                for tt in range(CT):
                    pv = psum.tile([128, 512], F32, tag="one", bufs=2)
                    for kt in range(KT):
                        nc.tensor.matmul(
                            pv[:, :],
                            hT_all[:, kt, j * 256 + tt * 128: j * 256 + tt * 128 + 128],
                            wq[:, (2 * KT + kt) * 512:(2 * KT + kt) * 512 + 512],
                            start=(kt == 0),
                            stop=(not with_bias and kt == KT - 1))
                    if with_bias:
                        nc.tensor.matmul(pv[:, :], ones_row[:, 0:128],
                                         brow_w(i, 0), start=False, stop=True)
                    nc.scalar.copy(
                        out=v_all[:, (j * CT + tt) * 512:(j * CT + tt) * 512 + 512],
                        in_=pv[:, :])

            pair_proj_phi(wq, 1, jp, kT_all, i, 1)

            kseqs = {}
            for j in (jp, jp + 1):
                # kseq[t%128, tt, e] = phi(k)[e, t] transposed
                kseq = work.tile([128, CT, 512], BF16, tag=f"kseq{j % 2}",
                                 bufs=1)
                kseqs[j] = kseq
                for ft in range(KT):
                    nc.sync.dma_start_transpose(
                        out=kseq[:, :, ft * 128:(ft + 1) * 128],
                        in_=kT_all[:, ft, j * 256:(j + 1) * 256])

            # delta state + prefix chain:  P[j+1] = P[j] + kseq^T [v|1]
            for j in (jp, jp + 1):
                kseq = kseqs[j]
                skp = psum.tile([128, KT], F32, tag="sm", bufs=2)
                for kt in range(KT):
                    pd = psum.tile([128, 512], F32, tag="one", bufs=2)
                    for tt in range(CT):
                        ks = kseq[:, tt, kt * 128:(kt + 1) * 128]
                        nc.tensor.matmul(
                            pd[:, :], ks,
                            v_all[:, (j * CT + tt) * 512:(j * CT + tt) * 512 + 512],
                            start=(tt == 0), stop=(tt == CT - 1))
                        nc.tensor.matmul(
                            skp[:, kt:kt + 1], ks, ones_col_bf,
                            start=(kt == 0 and tt == 0),
                            stop=(kt == KT - 1 and tt == CT - 1))
                    nc.vector.scalar_tensor_tensor(
                        out=Pb(j + 1, kt)[:, 0:E], in0=pd[:, :], scalar=1.0,
                        in1=Pb(j, kt)[:, 0:E], op0=ALU.mult, op1=ALU.add)
                for kt in range(KT):
                    nc.vector.scalar_tensor_tensor(
                        out=Pb(j + 1, kt)[:, E:SKW], in0=skp[:, kt:kt + 1], scalar=1.0,
                        in1=Pb(j, kt)[:, E:SKW], op0=ALU.mult, op1=ALU.add)

        def exchange():
            """Boundary-state AllReduce (pairwise)."""
            contrib = work.tile([128, KT * SKW], BF16, tag="contrib", bufs=1)
            nc.vector.tensor_scalar_mul(contrib,
                                        P_all[:, NCH * KT * SKW:(NCH + 1) * KT * SKW],
                                        mcol[:, 1:2])
            cc_out = dram.tile([128, KT * SKW], BF16, tag="cc_out")
            cc_in = dram.tile([128, KT * SKW], BF16, tag="cc_in")
            dma(out=cc_out, in_=contrib)
            if cc:
                nc.gpsimd.collective_compute(
                    "AllReduce", ALU.add, replica_groups=REPLICA_GROUPS,
                    ins=[cc_out.opt()], outs=[cc_in.opt()])
            else:
                nc.gpsimd.dma_start(out=cc_in.opt(), in_=cc_out.opt())
            dma(out=R_sb, in_=cc_in)

        def pass2a(i, wq):
            """q projection + masked intra-chunk scores (R-independent,
            overlaps the collective). The intra numerator/denominator
            matmuls are deferred to pass2b where they share the prefix
            PSUM accumulation groups."""
            mv2 = small.tile([128, 2 * NT8], F32, tag="mv2")
            qT_all = work.tile([128, KT, SH], BF16, tag="qTall", bufs=1)
            smks = {}
            for jp in range(0, NCH, 2):
                pair_proj_phi(wq, 0, jp, qT_all, i, 0)

                for j in (jp, jp + 1):
                    jo = j * 256
                    # scoresT: cols 0:256 = t0 x (s0|s1); cols 256:384 = t1 x s1
                    ps = psum.tile([128, 384], F32, tag="one", bufs=2)
                    for ft in range(KT):
                        nc.tensor.matmul(
                            ps[:, 0:256],
                            kT_all[:, ft, j * 256: j * 256 + 128],
                            qT_all[:, ft, jo: jo + 256],
                            start=(ft == 0), stop=False)
                        nc.tensor.matmul(
                            ps[:, 256:384],
                            kT_all[:, ft, j * 256 + 128: j * 256 + 256],
                            qT_all[:, ft, jo + 128: jo + 256],
                            start=False, stop=(ft == KT - 1))
                    sm = work.tile([128, 384], BF16, tag="smk", bufs=4)
                    smks[j] = sm
                    nc.vector.tensor_mul(sm[:, 0:128], ps[:, 0:128], tril)
                    nc.vector.tensor_copy(out=sm[:, 128:256], in_=ps[:, 128:256])
                    nc.vector.tensor_mul(sm[:, 256:384], ps[:, 256:384], tril)
            return mv2, qT_all, smks

        def make_pass2b(i, wo, mv2, qT_all, smks):
            """Helpers for prefix attention, chunk-granular."""
            if not no_r:
                nc.vector.tensor_scalar_mul(R_sb, R_sb, mcol[:, 0:1])
            Rloc = P_all[:, 0:KT * SKW] if no_r else R_sb
            peffs = {}
            for j in (1, 2, 3):
                peff = work.tile([128, KT * SKW], BF16, tag="peff", bufs=3)
                peffs[j] = peff
                nc.vector.tensor_tensor(
                    out=peff,
                    in0=P_all[:, j * KT * SKW:(j + 1) * KT * SKW],
                    in1=Rloc, op=ALU.add)

            state = {}

            def intra_seed(j):
                """R-independent seeds: intra numerator + denominator
                matmuls open the PSUM accumulation groups."""
                sm = smks[j]
                v0 = v_all[:, (j * CT + 0) * 512:(j * CT + 0) * 512 + 512]
                v1 = v_all[:, (j * CT + 1) * 512:(j * CT + 1) * 512 + 512]
                pn = psum.tile([128, 1024], F32, tag="big2", bufs=2)
                nc.tensor.matmul(pn[:, 0:512], sm[:, 0:128], v0,
                                 start=True, stop=False)
                nc.tensor.matmul(pn[:, 512:1024], sm[:, 128:256], v0,
                                 start=True, stop=False)
                nc.tensor.matmul(pn[:, 512:1024], sm[:, 256:384], v1,
                                 start=False, stop=False)
                pden = psum.tile([128, CT], F32, tag="sm", bufs=2)
                nc.tensor.matmul(pden[:, 0:1], sm[:, 0:128], ones_col_bf,
                                 start=True, stop=False)
                nc.tensor.matmul(pden[:, 1:2], sm[:, 128:256], ones_col_bf,
                                 start=False, stop=False)
                nc.tensor.matmul(pden[:, 1:2], sm[:, 256:384], ones_col_bf,
                                 start=False, stop=False)
                state[j] = (pn, pden)

            def prefix_fin(j):
                jo = j * 256
                pn, pden = state.pop(j)
                Peff = Rloc if j == 0 else peffs[j]
                for kt in range(KT):
                    nc.tensor.matmul(pn[:, 0:512],
                                     qT_all[:, kt, jo: jo + 128],
                                     Peff[:, kt * SKW: kt * SKW + E],
                                     start=False, stop=(kt == KT - 1))
                for kt in range(KT):
                    nc.tensor.matmul(pn[:, 512:1024],
                                     qT_all[:, kt, jo + 128: jo + 256],
                                     Peff[:, kt * SKW: kt * SKW + E],
                                     start=False, stop=(kt == KT - 1))
                for st in range(CT):
                    for kt in range(KT):
                        nc.tensor.matmul(
                            pden[:, st:st + 1],
                            qT_all[:, kt, jo + st * 128: jo + st * 128 + 128],
                            Peff[:, kt * SKW + E: kt * SKW + SKW],
                            start=False,
                            stop=(st == CT - 1 and kt == KT - 1))
                den = small.tile([128, CT], F32, tag="den")
                nc.vector.tensor_scalar_add(den, pden[:, :], EPS)
                rden = small.tile([128, CT], F32, tag="rden")
                nc.vector.reciprocal(out=rden, in_=den)

                # Unscaled num goes straight to the o-projection; 1/den is
                # applied at the residual (it commutes per-s). With biases
                # the scale must happen before adding b_o: scale here.
                attn = work.tile([128, CT * 512], BF16, tag="attnA", bufs=2)
                sc = (rden[:, 0:1], rden[:, 1:2]) if with_bias else (1.0, 1.0)
                nc.scalar.activation(out=attn[:, 0:512], in_=pn[:, 0:512],
                                     func=AF.Copy, scale=sc[0])
                nc.scalar.activation(out=attn[:, 512:1024], in_=pn[:, 512:1024],
                                     func=AF.Copy, scale=sc[1])
                attnT = work.tile([128, KT, 256], BF16, tag="attnT", bufs=2)
                for st in range(CT):
                    nc.sync.dma_start_transpose(
                        out=attnT[:, :, st * 128:(st + 1) * 128],
                        in_=attn[:, st * 512:(st + 1) * 512])
                state[j] = (attnT, rden)

            def oproj(j):
                attnT, rden = state.pop(j)
                for st in range(CT):
                    po = psum.tile([128, 512], F32, tag="one", bufs=2)
                    for mt in range(KT):
                        nc.tensor.matmul(po[:, :],
                                         attnT[:, mt, st * 128:(st + 1) * 128],
                                         wo[:, (0 * KT + mt) * 512:(0 * KT + mt) * 512 + 512],
                                         start=(mt == 0),
                                         stop=(not with_bias and mt == KT - 1))
                    if with_bias:
                        nc.tensor.matmul(po[:, :], ones_row[:, 0:128],
                                         brow_w(i, 1), start=False, stop=True)
                    ls = lat[:, (j * CT + st) * 512:(j * CT + st) * 512 + 512]
                    nc.vector.scalar_tensor_tensor(out=ls, in0=po[:, :],
                                                   scalar=(1.0 if with_bias
                                                           else rden[:, st:st + 1]),
                                                   in1=ls, op0=ALU.mult, op1=ALU.add)
                    ln_stats(mv2, j * CT + st, j * CT + st)

            def sweep2(j):
                """LN2 normalize + transpose for chunk j's two c-tiles."""
                nb2, rstd2 = ln_scales(mv2[:, j * CT * 2: j * CT * 2 + 4],
                                       2, "l2")
                for c2_ in range(CT):
                    ct8 = j * CT + c2_
                    y2 = work.tile([128, 512], BF16, tag="y2", bufs=2)
                    ln_norm(y2, ct8, nb2, rstd2, c2_)
                    nc.sync.dma_start_transpose(
                        out=h2T_all[:, :, ct8 * 128:(ct8 + 1) * 128],
                        in_=y2)

            return intra_seed, prefix_fin, oproj, sweep2

        def c1_chunk(i, j, wo):
            """FFN first projection + gelu for chunk j (rhs N=256)."""
            h1T = work.tile([128, KT, 256], BF16, tag="h1T", bufs=2)
            ph = psum.tile([128, 1024], F32, tag="big2", bufs=2)
            for ft in range(KT):
                for kt in range(KT):
                    nc.tensor.matmul(
                        ph[:, ft * 256:(ft + 1) * 256],
                        wo[:, (1 * KT + kt) * 512 + ft * 128:(1 * KT + kt) * 512 + ft * 128 + 128],
                        h2T_all[:, kt, j * 256:(j + 1) * 256],
                        start=(kt == 0), stop=(kt == KT - 1))
            for ft in range(KT):
                nc.scalar.activation(out=h1T[:, ft, :],
                                     in_=ph[:, ft * 256:(ft + 1) * 256],
                                     func=AF.Gelu, bias=bcol_w(i, 2, ft),
                                     scale=1.0)
            return h1T

        def c2_chunk(i, j, wo, h1T, mv1n):
            """FFN second projection + residual for chunk j; next layer's
            LN1 stats pipelined in when mv1n is given."""
            for st in range(CT):
                pf = psum.tile([128, 512], F32, tag="one", bufs=2)
                for mt in range(KT):
                    nc.tensor.matmul(
                        pf[:, :],
                        h1T[:, mt, st * 128:(st + 1) * 128],
                        wo[:, (2 * KT + mt) * 512:(2 * KT + mt) * 512 + 512],
                        start=(mt == 0),
                        stop=(not with_bias and mt == KT - 1))
                if with_bias:
                    nc.tensor.matmul(pf[:, :], ones_row[:, 0:128],
                                     brow_w(i, 2), start=False, stop=True)
                ls = lat[:, (j * CT + st) * 512:(j * CT + st) * 512 + 512]
                nc.vector.scalar_tensor_tensor(out=ls, in0=pf[:, :], scalar=1.0,
                                               in1=ls, op0=ALU.mult, op1=ALU.add)
                if mv1n is not None:
                    ln_stats(mv1n, j * CT + st, j * CT + st)

        # `repeat` unrolls the FULL computation (conv -> layers -> output
        # projection); every iteration recomputes the same output from x.
        # test.py uses repeat>1 to measure marginal per-iteration device
        # time through the fixed ~80ms launch overhead.
        for _rep in range(repeat):
            # ===== fused input conv (+ LN1 of layer 0 interleaved) =====
            _ph(nc, 'conv')
            wq, wo = load_weights(0)
            x_sb = work.tile([CIN, SH + 2], BF16, tag="x_sb", bufs=2)
            dma(out=x_sb, in_=x_d[:, :])
            mv1 = small.tile([128, 2 * NT8], F32, tag="mv1")
            for ct8 in range(NT8):
                pc = psum.tile([128, 512], F32, tag="one", bufs=2)
                for d in range(3):
                    nc.tensor.matmul(pc[:, :],
                                     x_sb[:, ct8 * 128 + d: ct8 * 128 + d + 128],
                                     cvWT[:, d * 512:(d + 1) * 512],
                                     start=(d == 0),
                                     stop=(not with_bias and d == 2))
                if with_bias:
                    for d in range(3):
                        nc.tensor.matmul(pc[:, :],
                                         halo[:, ct8 * 128 + d: ct8 * 128 + d + 128],
                                         cvb_row(d), start=False, stop=False)
                    nc.tensor.matmul(pc[:, :], ones_row[:, 0:128], ccb_row(),
                                     start=False, stop=True)
                nc.scalar.copy(out=lat[:, ct8 * 512:(ct8 + 1) * 512], in_=pc[:, :])
                ln_stats(mv1, ct8, ct8)
                if ct8 == 3:
                    ln1_sweep(mv1, 0)
                elif ct8 == 7:
                    ln1_sweep(mv1, 2)

            # ===== layer-0 pass 1 =====
            _ph(nc, 'L0_pass1')
            pass1_pair(0, 0, wq)
            pass1_pair(0, 2, wq)

            for i in range(L):
                last = (i + 1 == L)
                _ph(nc, f'L{i}_exch')
                exchange()
                _ph(nc, f'L{i}_pass2a')
                if not last:
                    wq_n, wo_n = load_weights(i + 1)
                mv2, qT_all, smks = pass2a(i, wq)
                _ph(nc, f'L{i}_pass2b')
                intra_seed, prefix_fin, oproj, sweep2 = \
                    make_pass2b(i, wo, mv2, qT_all, smks)
                if not last:
                    mv1n = small.tile([128, 2 * NT8], F32, tag="mv1")
                else:
                    mv1n = None

                # chunk-granular pipeline across pass2b + FFN
                intra_seed(0)
                intra_seed(1)
                prefix_fin(0)
                prefix_fin(1)
                intra_seed(2)
                oproj(0)
                prefix_fin(2)
                intra_seed(3)
                sweep2(0)
                oproj(1)
                prefix_fin(3)
                sweep2(1)
                h1T0 = c1_chunk(i, 0, wo)
                oproj(2)
                sweep2(2)
                h1T1 = c1_chunk(i, 1, wo)
                c2_chunk(i, 0, wo, h1T0, mv1n)
                oproj(3)
                sweep2(3)
                h1T2 = c1_chunk(i, 2, wo)
                c2_chunk(i, 1, wo, h1T1, mv1n)
                if not last:
                    ln1_sweep(mv1n, 0)
                _ph(nc, f'L{i}_pass3')
                c2_chunk(i, 2, wo, h1T2, mv1n)
                h1T3 = c1_chunk(i, 3, wo)
                if last:
                    latT = work.tile([128, KT, SH], BF16, tag="latT", bufs=1)
                    out_sb = work.tile([COUT, SH], F32, tag="out_sb", bufs=1)
                    for ct8 in range(4):
                        latb = work.tile([128, 512], BF16, tag="latb", bufs=2)
                        nc.scalar.copy(out=latb, in_=lat[:, ct8 * 512:(ct8 + 1) * 512])
                        nc.sync.dma_start_transpose(
                            out=latT[:, :, ct8 * 128:(ct8 + 1) * 128], in_=latb)
                c2_chunk(i, 3, wo, h1T3, mv1n)
                if not last:
                    ln1_sweep(mv1n, 2)
                    _ph(nc, f'L{i + 1}_pass1')
                    pass1_pair(i + 1, 0, wq_n)
                    pass1_pair(i + 1, 2, wq_n)
                    wq, wo = wq_n, wo_n
                else:
                    _ph(nc, 'tail')
                    for ct8 in range(4, NT8):
                        latb = work.tile([128, 512], BF16, tag="latb", bufs=2)
                        nc.scalar.copy(out=latb, in_=lat[:, ct8 * 512:(ct8 + 1) * 512])
                        nc.sync.dma_start_transpose(
                            out=latT[:, :, ct8 * 128:(ct8 + 1) * 128], in_=latb)
                    for sb in range(SH // 512):
                        pout = psum.tile([COUT, 512], F32, tag="one", bufs=2)
                        for kt in range(KT):
                            nc.tensor.matmul(pout[:, :],
                                             outWT[:, kt * COUT:(kt + 1) * COUT],
                                             latT[:, kt, sb * 512:(sb + 1) * 512],
                                             start=(kt == 0),
                                             stop=(not with_bias and kt == KT - 1))
                        if with_bias:
                            nc.tensor.matmul(pout[:, :], outb_row(), ones_row,
                                             start=False, stop=True)
                        nc.scalar.copy(out=out_sb[:, sb * 512:(sb + 1) * 512],
                                       in_=pout[:, :])
                    dma(out=out_d[:, :], in_=out_sb)


# ---------------- host side ----------------

_CACHE = threading.local()


def _get_program(with_bias=False, loop_input=False):
    key = f"nc_{with_bias}_{loop_input}"
    if not hasattr(_CACHE, key):
        setattr(_CACHE, key, build_program(with_bias=with_bias,
                                           loop_input=loop_input))
    return getattr(_CACHE, key)


def _needs_bias(inputs):
    f32 = np.float32
    ln1_b = np.asarray(inputs["ln1_b"], f32)
    ln2_b = np.asarray(inputs["ln2_b"], f32)
    vals = [np.asarray(inputs[k], f32) for k in
            ("in_b", "cc_b", "out_b", "bo", "c2_b")]
    bv_eff = np.asarray(inputs["bv"], f32) + np.einsum(
        "loe,le->lo", np.asarray(inputs["Wv"], f32), ln1_b)
    vals.append(bv_eff)
    return any(np.abs(v).max() > 0 for v in vals)


def _prep_shared(inputs):
    f32 = np.float32
    inW = np.asarray(inputs["in_W"], f32)      # [E, CIN]
    in_b = np.asarray(inputs["in_b"], f32)
    ccW = np.asarray(inputs["cc_W"], f32)      # [E, E, 3]
    cc_b = np.asarray(inputs["cc_b"], f32)
    outW = np.asarray(inputs["out_W"], f32)    # [COUT, E]
    out_b = np.asarray(inputs["out_b"], f32)

    # fused conv: V_d = ccW[:,:,d] @ inW  -> [E, CIN]; store V_d^T
    cvWT = np.zeros((CIN, 3 * E), f32)
    for d in range(3):
        cvWT[:, d * E:(d + 1) * E] = (ccW[:, :, d] @ inW).T

    ln1_g = np.asarray(inputs["ln1_g"], f32); ln1_b = np.asarray(inputs["ln1_b"], f32)
    ln2_g = np.asarray(inputs["ln2_g"], f32); ln2_b = np.asarray(inputs["ln2_b"], f32)

    wpack = np.zeros((128, L * 6 * KT * 512), f32)
    brow = np.zeros((1, BROW_N), f32)
    bcol = np.zeros((128, L * 3 * KT), f32)
    for i in range(L):
        biases = {}
        for w, (Wn, bn, g, bb) in enumerate((
                ("Wq", "bq", ln1_g[i], ln1_b[i]),
                ("Wk", "bk", ln1_g[i], ln1_b[i]),
                ("Wv", "bv", ln1_g[i], ln1_b[i]),
                ("Wo", "bo", None, None),
                ("c1_W", "c1_b", ln2_g[i], ln2_b[i]),
                ("c2_W", "c2_b", None, None))):
            W = np.asarray(inputs[Wn], f32)[i]          # [E_out, E_in]
            bias = np.asarray(inputs[bn], f32)[i].copy()
            if g is not None:
                WT = (W * g[None, :]).T                  # fold LN gain
                bias = bias + W @ bb                     # fold LN bias
            else:
                WT = W.T
            for kt in range(KT):
                wpack[:, (i * 6 + w) * KT * 512 + kt * 512:
                      (i * 6 + w) * KT * 512 + kt * 512 + 512] = \
                    WT[kt * 128:(kt + 1) * 128, :]
            biases[w] = bias
        # rows: bv, bo, c2b
        brow[0, (i * 3 + 0) * 512:(i * 3 + 0) * 512 + 512] = biases[2]
        brow[0, (i * 3 + 1) * 512:(i * 3 + 1) * 512 + 512] = biases[3]
        brow[0, (i * 3 + 2) * 512:(i * 3 + 2) * 512 + 512] = biases[5]
        # cols: bq, bk, c1b
        for which, w in ((0, 0), (1, 1), (2, 4)):
            for kt in range(KT):
                bcol[:, (i * 3 + which) * KT + kt] = biases[w][kt * 128:(kt + 1) * 128]

    outWT = np.zeros((128, KT * COUT), f32)
    for kt in range(KT):
        outWT[:, kt * COUT:(kt + 1) * COUT] = outW.T[kt * 128:(kt + 1) * 128, :]

    # conv-bias rows: (W_d @ in_b) masked by halo at use time; then cc_b, out_b
    for d in range(3):
        brow[0, L * 3 * 512 + d * 512: L * 3 * 512 + (d + 1) * 512] = \
            ccW[:, :, d] @ in_b
    brow[0, L * 3 * 512 + 3 * 512: L * 3 * 512 + 4 * 512] = cc_b
    brow[0, L * 3 * 512 + 4 * 512: L * 3 * 512 + 4 * 512 + COUT] = out_b

    tril = np.tril(np.ones((128, 128), f32)).T  # keep t<=s in [t,s] layout

    return {
        "cvWT": cvWT.astype(BF),
        "wpack": wpack.astype(BF),
        "outWT": outWT.astype(BF),
        "brow": brow.astype(BF),
        "bcol": bcol,
        "tril": tril.astype(BF),
        "ones_row": np.ones((1, 512), f32).astype(BF),
        "ones_col_bf": np.ones((128, 1), f32).astype(BF),
    }


def _prep_core_inputs(shared, inputs, b, h):
    f32 = np.float32
    x = np.asarray(inputs["x"], f32)
    s0 = h * SH
    x_sl = np.zeros((CIN, SH + 2), f32)
    lo = max(0, s0 - 2)
    x_sl[:, 2 - (s0 - lo):] = x[b, :, lo:s0 + SH]
    halo = np.ones((1, SH + 2), f32)
    if h == 0:
        halo[0, :2] = 0.0
    mcol = np.zeros((128, 2), f32)
    mcol[:, 0] = float(h)
    mcol[:, 1] = 1.0 - float(h)
    m = dict(shared)
    m["x_sl"] = x_sl.astype(BF)
    m["halo"] = halo.astype(BF)
    m["mcol"] = mcol
    return m


def _run(inputs, loop_input=False, rept=1, **kw):
    nc = _get_program(with_bias=_needs_bias(inputs), loop_input=loop_input)
    shared = _prep_shared(inputs)
    in_maps = []
    for core in range(NCORES):
        b, h = core // 2, core % 2
        m = _prep_core_inputs(shared, inputs, b, h)
        if loop_input:
            m["rept"] = np.full((1, 1), rept, np.int32)
        in_maps.append(m)
    return run_bass_kernel_spmd(nc, in_maps, core_ids=list(range(NCORES)), **kw)


def kernel(**inputs):
    res = _run(inputs)
    out = np.zeros((B, COUT, S), np.float32)
    for core in range(NCORES):
        b, h = core // 2, core % 2
        out[b, :, h * SH:(h + 1) * SH] = res.results[core]["out"]
    return out


def bench(inputs, trace_cores=(0, 1), tmpdir=None):
    """Run with NTFF tracing; returns BassKernelResults with exec_time_ns."""
    return _run(inputs, trace=True, trace_cores=list(trace_cores), tmpdir=tmpdir)


# revision 31
# speedup vs baseline: 1.0636x; 1.0636x over previous
"""Recurrent linear-attention transformer on 8 Trainium2 NeuronCores.

Sharding: 4-way data parallel over batch x 2-way sequence split (halves of
S=2048). Causal linear attention is computed in chunked form; the only
cross-core dependency is the cumulative (k^T v, sum k) state at the half
boundary, exchanged once per layer via a pairwise AllReduce.

Schedule (v2, PE-density focused):
  - The input projection is folded into the causal conv on the host
    (V_d = ccW[:,:,d] @ in_W), so the conv contracts over CIN=64 only:
    3 matmuls per seq tile instead of 15.
  - LN1 for layer 0 is interleaved with the conv tiles.
  - Per layer: pass1 (v then k projections + per-chunk prefix states) ->
    boundary AllReduce -> pass2a (q projection + masked intra scores,
    overlapping the collective) -> Peff = P_local + R adds on DVE ->
    a chunk-granular pipeline over pass2b + FFN: each chunk's intra
    numerator/denominator matmuls SEED the PSUM accumulation groups
    before R arrives, the prefix matmuls accumulate on top (no identity
    re-injection), and o-projection / LN2 sweep / FFN c1 (N=256 per
    chunk) / c2 are interleaved so the attn-transpose and LN chains hide
    under PE work across the layer boundary into the next pass1.
  - Weight DMAs for layer i+1 are emitted at the top of pass2a(i): DMA
    queues are round-robin in emission order with head-of-line blocking,
    so late-emitted weight loads would queue behind dependency-stalled
    transposes.
  - LN scales use sqrt(var+eps) on ACT + reciprocal on DVE; normalize
    runs on ACT (Identity with per-partition scale/bias) to keep the
    DVE queue shallow. PSUM: tag "one" x3, "big2" (1024) x2, "sm" x1
    = 8 banks; one start per PSUM zero-region per accumulation group.
    All projection matmuls (k/q/v) draw PSUM from the deep "one" pool;
    the 2-buffer "big2" rotation serves only the attention-numerator and
    FFN-c1 groups — PSUM-drain slack is the dominant hardware lever that
    the cost-model simulator does not see (bufs=2 -> 3 on the drain tiles
    measured ~45us, this split another ~25us).

Per-core layout conventions (SBUF 2-D tensors, 128 partitions):
  lat   f32  [128, 8*512]    seq-major residual: c-tile ct8 block, cols = e
  hT    bf16 [128, 4*4*256]  feature-major LN1 output: (chunk, kt) blocks
  kT    bf16 [128, 4*4*256]  phi(k) feature-major: (chunk, ft) blocks, cols=t
  v     bf16 [128, 4*2*512]  v seq-major: (chunk, tt) blocks, cols = e'
  P_all bf16 [128, 5*4*513]  prefix states: (j, kt) -> [KV[e,e'] | s_k[e]]
All matmul operands bf16, PSUM/stats/residual f32.

`build_program(repeat=R)` unrolls the whole computation R times (identical
output each iteration); test.py uses it to measure per-iteration device time
through the ~80 ms fixed launch overhead of the axon-tunneled PJRT path.
"""

import threading

import numpy as np
import ml_dtypes

import concourse.bass as bass
import concourse.bacc as bacc
import concourse.tile as tile
import concourse.mybir as mybir
from concourse.bass_utils import run_bass_kernel_spmd

AF = mybir.ActivationFunctionType
ALU = mybir.AluOpType
F32 = mybir.dt.float32
BF16 = mybir.dt.bfloat16
BF = ml_dtypes.bfloat16

L, B, CIN, COUT, E, S = 4, 4, 64, 64, 512, 2048
NCORES = 8
SH = S // 2          # per-core sequence half
C = 256              # attention chunk
NCH = SH // C        # 4 chunks
CT = C // 128        # 2 c-tiles per chunk
KT = E // 128        # 4 feature tiles
NT8 = SH // 128      # 8 seq tiles per half
EPS = 1e-6
LN_EPS = 1e-5
SKW = E + 1          # 513: KV block plus s_k column

REPLICA_GROUPS = [[0, 1], [2, 3], [4, 5], [6, 7]]

PHASE_HOOK = None  # optional (name, next_instr_id) recorder for profiling


def _ph(nc, name):
    if PHASE_HOOK is not None:
        PHASE_HOOK(name, nc.next_id())

# brow layout (bf16): per-layer [bv, bo, c2b] rows, then 3 conv-bias rows
# (W_d @ in_b), cc_b, out_b
BROW_N = L * 3 * 512 + 3 * 512 + 512 + COUT


def build_program(cc=True, repeat=1, with_bias=False, loop_input=False, no_r=False):
    nc = bacc.Bacc("TRN2", target_bir_lowering=False, debug=False,
                   num_devices=NCORES)

    rept_d = (nc.dram_tensor("rept", [1, 1], mybir.dt.int32,
                             kind="ExternalInput") if loop_input else None)
    x_d = nc.dram_tensor("x_sl", [CIN, SH + 2], BF16, kind="ExternalInput")
    cvWT_d = nc.dram_tensor("cvWT", [CIN, 3 * E], BF16, kind="ExternalInput")
    wpack_d = nc.dram_tensor("wpack", [128, L * 6 * KT * 512], BF16, kind="ExternalInput")
    outWT_d = nc.dram_tensor("outWT", [128, KT * COUT], BF16, kind="ExternalInput")
    brow_d = nc.dram_tensor("brow", [1, BROW_N], BF16, kind="ExternalInput")
    bcol_d = nc.dram_tensor("bcol", [128, L * 3 * KT], F32, kind="ExternalInput")
    tril_d = nc.dram_tensor("tril", [128, 128], BF16, kind="ExternalInput")
    mcol_d = nc.dram_tensor("mcol", [128, 2], F32, kind="ExternalInput")  # [m, 1-m]
    halo_d = nc.dram_tensor("halo", [1, SH + 2], BF16, kind="ExternalInput")
    ones_row_d = nc.dram_tensor("ones_row", [1, 512], BF16, kind="ExternalInput")
    ones_col_bf_d = nc.dram_tensor("ones_col_bf", [128, 1], BF16, kind="ExternalInput")

    out_d = nc.dram_tensor("out", [COUT, SH], F32, kind="ExternalOutput")

    with tile.TileContext(nc, num_cores=NCORES) as tc:
        _emit(nc, tc, x_d, cvWT_d, wpack_d, outWT_d, brow_d, bcol_d,
              tril_d, mcol_d, halo_d, ones_row_d,
              ones_col_bf_d, out_d, cc=cc, repeat=repeat,
              with_bias=with_bias, rept_d=rept_d, no_r=no_r)
    nc.compile()
    return nc


def _emit(nc, tc, x_d, cvWT_d, wpack_d, outWT_d, brow_d, bcol_d,
          tril_d, mcol_d, halo_d, ones_row_d,
          ones_col_bf_d, out_d, cc=True, repeat=1,
          with_bias=False, rept_d=None, no_r=False):
    import contextlib
    ctx = contextlib.ExitStack()
    with ctx:
        singles = ctx.enter_context(tc.tile_pool(name="singles", bufs=1))
        persist = ctx.enter_context(tc.tile_pool(name="persist", bufs=1))
        small = ctx.enter_context(tc.tile_pool(name="small", bufs=4))
        psum = ctx.enter_context(tc.tile_pool(name="psum", bufs=2, space="PSUM"))
        dram = ctx.enter_context(tc.tile_pool(name="dram", bufs=2, space="DRAM"))

        dma = nc.sync.dma_start

        # ---- constants ----
        outWT = singles.tile([128, KT * COUT], BF16)
        dma(out=outWT, in_=outWT_d[:, :])
        cvWT = singles.tile([CIN, 3 * E], BF16)
        dma(out=cvWT, in_=cvWT_d[:, :])
        bcol = singles.tile([128, L * 3 * KT], F32)
        dma(out=bcol, in_=bcol_d[:, :])
        tril = singles.tile([128, 128], BF16)
        dma(out=tril, in_=tril_d[:, :])
        mcol = singles.tile([128, 2], F32)
        dma(out=mcol, in_=mcol_d[:, :])
        ones_col_bf = singles.tile([128, 1], BF16)
        dma(out=ones_col_bf, in_=ones_col_bf_d[:, :])
        eps_col = singles.tile([128, 1], F32)
        nc.vector.memset(eps_col, LN_EPS)
        if with_bias:
            brow = singles.tile([1, BROW_N], BF16)
            dma(out=brow, in_=brow_d[:, :])
            halo = singles.tile([1, SH + 2], BF16)
            dma(out=halo, in_=halo_d[:, :])
            ones_row = singles.tile([1, 512], BF16)
            dma(out=ones_row, in_=ones_row_d[:, :])

        cvb_row = lambda d: brow[:, L * 3 * 512 + d * 512: L * 3 * 512 + (d + 1) * 512]
        ccb_row = lambda: brow[:, L * 3 * 512 + 3 * 512: L * 3 * 512 + 4 * 512]
        outb_row = lambda: brow[:, L * 3 * 512 + 4 * 512: L * 3 * 512 + 4 * 512 + COUT]

        def brow_w(i, w):
            # w: 0=bv 1=bo 2=c2b
            return brow[:, (i * 3 + w) * 512:(i * 3 + w) * 512 + 512]

        def bcol_w(i, which, kt):
            # which: 0=bq 1=bk 2=c1b
            c = (i * 3 + which) * KT + kt
            return bcol[:, c:c + 1]

        # ---- persistent state ----
        lat = persist.tile([128, NT8 * 512], F32)
        P_all = persist.tile([128, (NCH + 1) * KT * SKW], BF16)
        nc.vector.memset(P_all[:, 0:KT * SKW], 0.0)
        # feature-tile-major activations: [within-tile row, tile, seq col]
        hT_all = persist.tile([128, KT, SH // 4 * 4], BF16)   # [e%128, kt, s]
        kT_all = persist.tile([128, KT, SH // 4 * 4], BF16)   # [feat%128, ft, t]
        h2T_all = persist.tile([128, KT, SH // 4 * 4], BF16)
        v_all = persist.tile([128, NCH * CT * 512], BF16)
        R_sb = persist.tile([128, KT * SKW], BF16)

        def Pb(j, kt):
            o = (j * KT + kt) * SKW
            return P_all[:, o:o + SKW]

        # layer-loop pools (persist across repeats)
        wbufs = 1 if with_bias else 2
        wq_pool = ctx.enter_context(tc.tile_pool(name="wq_pool", bufs=wbufs))
        wo_pool = ctx.enter_context(tc.tile_pool(name="wo_pool", bufs=wbufs))
        work = ctx.enter_context(tc.tile_pool(name="work", bufs=2))

        def load_weights(i):
            """DMA layer-i weights; returns (wq, wo) tiles."""
            wq = wq_pool.tile([128, 3 * KT * 512], BF16, tag="wq")
            wo = wo_pool.tile([128, 3 * KT * 512], BF16, tag="wo")
            for wti in range(3):
                dma(out=wq[:, wti * KT * 512:(wti + 1) * KT * 512],
                    in_=wpack_d[:, (i * 6 + wti) * KT * 512:(i * 6 + wti + 1) * KT * 512])
                dma(out=wo[:, wti * KT * 512:(wti + 1) * KT * 512],
                    in_=wpack_d[:, (i * 6 + 3 + wti) * KT * 512:(i * 6 + 4 + wti) * KT * 512])
            return wq, wo

        # ---------- layernorm helpers ----------
        def ln_stats(mv8, ct8, sl):
            """bn stats of lat c-tile ct8 into mv8[:, 2*sl:2*sl+2]."""
            stats = small.tile([128, 6], F32, tag="lnst")
            nc.vector.bn_stats(out=stats, in_=lat[:, ct8 * 512:(ct8 + 1) * 512])
            nc.vector.bn_aggr(out=mv8[:, 2 * sl:2 * sl + 2], in_=stats)

        def ln_scales(mv8, n, tag):
            """From interleaved [mean,var] pairs build rstd [128,n] and
            nb = -mean*rstd [128,n] (ACT sqrt + DVE reciprocal)."""
            sd = small.tile([128, n], F32, tag=tag + "sd")
            rstd = small.tile([128, n], F32, tag=tag + "rs")
            nb = small.tile([128, n], F32, tag=tag + "nb")
            mvv = mv8[:, 0:2 * n].rearrange("p (n two) -> p n two", two=2)
            nc.scalar.activation(out=sd, in_=mvv[:, :, 1], func=AF.Sqrt,
                                 bias=eps_col[:, 0:1], scale=1.0)
            nc.vector.reciprocal(out=rstd, in_=sd)
            nc.vector.scalar_tensor_tensor(out=nb, in0=mvv[:, :, 0], scalar=-1.0,
                                           in1=rstd, op0=ALU.mult, op1=ALU.mult)
            return nb, rstd

        def ln_norm(dst_y, ct8, nb, rstd, sl):
            nc.scalar.activation(out=dst_y, in_=lat[:, ct8 * 512:(ct8 + 1) * 512],
                                 func=AF.Identity, bias=nb[:, sl:sl + 1],
                                 scale=rstd[:, sl:sl + 1])

        def ln1_sweep(mv, jp):
            """Per-pair LN1 finish: rsqrt batch, normalize, transpose to hT."""
            nb, rstd = ln_scales(mv[:, jp * CT * 2: jp * CT * 2 + 8], 4, "l1")
            for c4 in range(4):
                y = work.tile([128, 512], BF16, tag="y", bufs=2)
                ln_norm(y, jp * CT + c4, nb, rstd, c4)
                nc.sync.dma_start_transpose(
                    out=hT_all[:, :, jp * 256 + c4 * 128: jp * 256 + c4 * 128 + 128],
                    in_=y)

        def pair_proj_phi(wt, w, jp, dst3, i, which):
            """Feature-major projection for a chunk pair with phi applied.
            dst3: 3D [128, KT, SH] tile written at cols [jp*256, jp*256+512).
            PSUM comes from the deep 'one' pool so the big2 rotation stays
            exclusive to the attention-numerator / FFN-c1 pipeline."""
            for fh in range(2):
                pps = []
                for fi in range(2):
                    pp = psum.tile([128, 512], F32, tag="one", bufs=3)
                    pps.append(pp)
                    ft = fh * 2 + fi
                    for kt in range(KT):
                        nc.tensor.matmul(
                            pp[:, :],
                            wt[:, (w * KT + kt) * 512 + ft * 128:(w * KT + kt) * 512 + ft * 128 + 128],
                            hT_all[:, kt, jp * 256: jp * 256 + 512],
                            start=(kt == 0), stop=(kt == KT - 1))
                et = work.tile([128, 1024], BF16, tag="phiE")
                for fi in range(2):
                    ft = fh * 2 + fi
                    bc = bcol_w(i, which, ft)
                    nc.scalar.activation(out=et[:, fi * 512:(fi + 1) * 512],
                                         in_=pps[fi][:, :],
                                         func=AF.Exp, bias=bc, scale=1.0)
                    nc.scalar.activation(out=dst3[:, ft, jp * 256: jp * 256 + 512],
                                         in_=pps[fi][:, :],
                                         func=AF.Relu, bias=bc, scale=1.0)
                d = dst3[:, fh * 2: fh * 2 + 2, jp * 256: jp * 256 + 512]
                nc.vector.scalar_tensor_tensor(out=d, in0=et, scalar=1.0, in1=d,
                                               op0=ALU.min, op1=ALU.add)

        def pass1_pair(i, jp, wq):
            """k/v projections + prefix-state chain for chunks jp, jp+1.
            v matmuls are emitted before the k projection so their PSUM
            drains are not queued behind the phi activations."""
            for j in (jp, jp + 1):
                for tt in range(CT):
                    pv = psum.tile([128, 512], F32, tag="one", bufs=2)
                    for kt in range(KT):
                        nc.tensor.matmul(
                            pv[:, :],
                            hT_all[:, kt, j * 256 + tt * 128: j * 256 + tt * 128 + 128],
                            wq[:, (2 * KT + kt) * 512:(2 * KT + kt) * 512 + 512],
                            start=(kt == 0),
                            stop=(not with_bias and kt == KT - 1))
                    if with_bias:
                        nc.tensor.matmul(pv[:, :], ones_row[:, 0:128],
                                         brow_w(i, 0), start=False, stop=True)
                    nc.scalar.copy(
                        out=v_all[:, (j * CT + tt) * 512:(j * CT + tt) * 512 + 512],
                        in_=pv[:, :])

            pair_proj_phi(wq, 1, jp, kT_all, i, 1)

            kseqs = {}
            for j in (jp, jp + 1):
                # kseq[t%128, tt, e] = phi(k)[e, t] transposed
                kseq = work.tile([128, CT, 512], BF16, tag=f"kseq{j % 2}",
                                 bufs=1)
                kseqs[j] = kseq
                for ft in range(KT):
                    nc.sync.dma_start_transpose(
                        out=kseq[:, :, ft * 128:(ft + 1) * 128],
                        in_=kT_all[:, ft, j * 256:(j + 1) * 256])

            # delta state + prefix chain:  P[j+1] = P[j] + kseq^T [v|1]
            for j in (jp, jp + 1):
                kseq = kseqs[j]
                skp = psum.tile([128, KT], F32, tag="sm", bufs=2)
                for kt in range(KT):
                    pd = psum.tile([128, 512], F32, tag="one", bufs=2)
                    for tt in range(CT):
                        ks = kseq[:, tt, kt * 128:(kt + 1) * 128]
                        nc.tensor.matmul(
                            pd[:, :], ks,
                            v_all[:, (j * CT + tt) * 512:(j * CT + tt) * 512 + 512],
                            start=(tt == 0), stop=(tt == CT - 1))
                        nc.tensor.matmul(
                            skp[:, kt:kt + 1], ks, ones_col_bf,
                            start=(kt == 0 and tt == 0),
                            stop=(kt == KT - 1 and tt == CT - 1))
                    nc.vector.scalar_tensor_tensor(
                        out=Pb(j + 1, kt)[:, 0:E], in0=pd[:, :], scalar=1.0,
                        in1=Pb(j, kt)[:, 0:E], op0=ALU.mult, op1=ALU.add)
                for kt in range(KT):
                    nc.vector.scalar_tensor_tensor(
                        out=Pb(j + 1, kt)[:, E:SKW], in0=skp[:, kt:kt + 1], scalar=1.0,
                        in1=Pb(j, kt)[:, E:SKW], op0=ALU.mult, op1=ALU.add)

        def exchange():
            """Boundary-state AllReduce (pairwise)."""
            contrib = work.tile([128, KT * SKW], BF16, tag="contrib", bufs=1)
            nc.vector.tensor_scalar_mul(contrib,
                                        P_all[:, NCH * KT * SKW:(NCH + 1) * KT * SKW],
                                        mcol[:, 1:2])
            cc_out = dram.tile([128, KT * SKW], BF16, tag="cc_out")
            cc_in = dram.tile([128, KT * SKW], BF16, tag="cc_in")
            dma(out=cc_out, in_=contrib)
            if cc:
                nc.gpsimd.collective_compute(
                    "AllReduce", ALU.add, replica_groups=REPLICA_GROUPS,
                    ins=[cc_out.opt()], outs=[cc_in.opt()])
            else:
                nc.gpsimd.dma_start(out=cc_in.opt(), in_=cc_out.opt())
            dma(out=R_sb, in_=cc_in)

        def pass2a(i, wq):
            """q projection + masked intra-chunk scores (R-independent,
            overlaps the collective). The intra numerator/denominator
            matmuls are deferred to pass2b where they share the prefix
            PSUM accumulation groups."""
            mv2 = small.tile([128, 2 * NT8], F32, tag="mv2")
            qT_all = work.tile([128, KT, SH], BF16, tag="qTall", bufs=1)
            smks = {}
            for jp in range(0, NCH, 2):
                pair_proj_phi(wq, 0, jp, qT_all, i, 0)

                for j in (jp, jp + 1):
                    jo = j * 256
                    # scoresT: cols 0:256 = t0 x (s0|s1); cols 256:384 = t1 x s1
                    ps = psum.tile([128, 384], F32, tag="one", bufs=2)
                    for ft in range(KT):
                        nc.tensor.matmul(
                            ps[:, 0:256],
                            kT_all[:, ft, j * 256: j * 256 + 128],
                            qT_all[:, ft, jo: jo + 256],
                            start=(ft == 0), stop=False)
                        nc.tensor.matmul(
                            ps[:, 256:384],
                            kT_all[:, ft, j * 256 + 128: j * 256 + 256],
                            qT_all[:, ft, jo + 128: jo + 256],
                            start=False, stop=(ft == KT - 1))
                    sm = work.tile([128, 384], BF16, tag="smk", bufs=4)
                    smks[j] = sm
                    nc.vector.tensor_mul(sm[:, 0:128], ps[:, 0:128], tril)
                    nc.vector.tensor_copy(out=sm[:, 128:256], in_=ps[:, 128:256])
                    nc.vector.tensor_mul(sm[:, 256:384], ps[:, 256:384], tril)
            return mv2, qT_all, smks

        def make_pass2b(i, wo, mv2, qT_all, smks):
            """Helpers for prefix attention, chunk-granular."""
            if not no_r:
                nc.vector.tensor_scalar_mul(R_sb, R_sb, mcol[:, 0:1])
            Rloc = P_all[:, 0:KT * SKW] if no_r else R_sb
            peffs = {}
            for j in (1, 2, 3):
                peff = work.tile([128, KT * SKW], BF16, tag="peff", bufs=3)
                peffs[j] = peff
                nc.vector.tensor_tensor(
                    out=peff,
                    in0=P_all[:, j * KT * SKW:(j + 1) * KT * SKW],
                    in1=Rloc, op=ALU.add)

            state = {}

            def intra_seed(j):
                """R-independent seeds: intra numerator + denominator
                matmuls open the PSUM accumulation groups."""
                sm = smks[j]
                v0 = v_all[:, (j * CT + 0) * 512:(j * CT + 0) * 512 + 512]
                v1 = v_all[:, (j * CT + 1) * 512:(j * CT + 1) * 512 + 512]
                pn = psum.tile([128, 1024], F32, tag="big2", bufs=2)
                nc.tensor.matmul(pn[:, 0:512], sm[:, 0:128], v0,
                                 start=True, stop=False)
                nc.tensor.matmul(pn[:, 512:1024], sm[:, 128:256], v0,
                                 start=True, stop=False)
                nc.tensor.matmul(pn[:, 512:1024], sm[:, 256:384], v1,
                                 start=False, stop=False)
                pden = psum.tile([128, CT], F32, tag="sm", bufs=2)
                nc.tensor.matmul(pden[:, 0:1], sm[:, 0:128], ones_col_bf,
                                 start=True, stop=False)
                nc.tensor.matmul(pden[:, 1:2], sm[:, 128:256], ones_col_bf,
                                 start=False, stop=False)
                nc.tensor.matmul(pden[:, 1:2], sm[:, 256:384], ones_col_bf,
                                 start=False, stop=False)
                state[j] = (pn, pden)

            def prefix_fin(j):
                jo = j * 256
                pn, pden = state.pop(j)
                Peff = Rloc if j == 0 else peffs[j]
                for kt in range(KT):
                    nc.tensor.matmul(pn[:, 0:512],
                                     qT_all[:, kt, jo: jo + 128],
                                     Peff[:, kt * SKW: kt * SKW + E],
                                     start=False, stop=(kt == KT - 1))
                for kt in range(KT):
                    nc.tensor.matmul(pn[:, 512:1024],
                                     qT_all[:, kt, jo + 128: jo + 256],
                                     Peff[:, kt * SKW: kt * SKW + E],
                                     start=False, stop=(kt == KT - 1))
                for st in range(CT):
                    for kt in range(KT):
                        nc.tensor.matmul(
                            pden[:, st:st + 1],
                            qT_all[:, kt, jo + st * 128: jo + st * 128 + 128],
                            Peff[:, kt * SKW + E: kt * SKW + SKW],
                            start=False,
                            stop=(st == CT - 1 and kt == KT - 1))
                den = small.tile([128, CT], F32, tag="den")
                nc.vector.tensor_scalar_add(den, pden[:, :], EPS)
                rden = small.tile([128, CT], F32, tag="rden")
                nc.vector.reciprocal(out=rden, in_=den)

                # Unscaled num goes straight to the o-projection; 1/den is
                # applied at the residual (it commutes per-s). With biases
                # the scale must happen before adding b_o: scale here.
                attn = work.tile([128, CT * 512], BF16, tag="attnA", bufs=2)
                sc = (rden[:, 0:1], rden[:, 1:2]) if with_bias else (1.0, 1.0)
                nc.scalar.activation(out=attn[:, 0:512], in_=pn[:, 0:512],
                                     func=AF.Copy, scale=sc[0])
                nc.scalar.activation(out=attn[:, 512:1024], in_=pn[:, 512:1024],
                                     func=AF.Copy, scale=sc[1])
                attnT = work.tile([128, KT, 256], BF16, tag="attnT", bufs=2)
                for st in range(CT):
                    nc.sync.dma_start_transpose(
                        out=attnT[:, :, st * 128:(st + 1) * 128],
                        in_=attn[:, st * 512:(st + 1) * 512])
                state[j] = (attnT, rden)

            def oproj(j):
                attnT, rden = state.pop(j)
                for st in range(CT):
                    po = psum.tile([128, 512], F32, tag="one", bufs=2)
                    for mt in range(KT):
                        nc.tensor.matmul(po[:, :],
                                         attnT[:, mt, st * 128:(st + 1) * 128],
                                         wo[:, (0 * KT + mt) * 512:(0 * KT + mt) * 512 + 512],
                                         start=(mt == 0),
                                         stop=(not with_bias and mt == KT - 1))
                    if with_bias:
                        nc.tensor.matmul(po[:, :], ones_row[:, 0:128],
                                         brow_w(i, 1), start=False, stop=True)
                    ls = lat[:, (j * CT + st) * 512:(j * CT + st) * 512 + 512]
                    nc.vector.scalar_tensor_tensor(out=ls, in0=po[:, :],
                                                   scalar=(1.0 if with_bias
                                                           else rden[:, st:st + 1]),
                                                   in1=ls, op0=ALU.mult, op1=ALU.add)
                    ln_stats(mv2, j * CT + st, j * CT + st)

            def sweep2(j):
                """LN2 normalize + transpose for chunk j's two c-tiles."""
                nb2, rstd2 = ln_scales(mv2[:, j * CT * 2: j * CT * 2 + 4],
                                       2, "l2")
                for c2_ in range(CT):
                    ct8 = j * CT + c2_
                    y2 = work.tile([128, 512], BF16, tag="y2", bufs=2)
                    ln_norm(y2, ct8, nb2, rstd2, c2_)
                    nc.sync.dma_start_transpose(
                        out=h2T_all[:, :, ct8 * 128:(ct8 + 1) * 128],
                        in_=y2)

            return intra_seed, prefix_fin, oproj, sweep2

        def c1_chunk(i, j, wo):
            """FFN first projection + gelu for chunk j (rhs N=256). PSUM
            from the deep 'one' pool (two single-start 512-col groups) so
            big2 stays a dedicated double-buffer for the attention
            numerators."""
            h1T = work.tile([128, KT, 256], BF16, tag="h1T", bufs=2)
            for fh in range(2):
                ph = psum.tile([128, 512], F32, tag="one", bufs=3)
                for fi in range(2):
                    ft = fh * 2 + fi
                    for kt in range(KT):
                        nc.tensor.matmul(
                            ph[:, fi * 256:(fi + 1) * 256],
                            wo[:, (1 * KT + kt) * 512 + ft * 128:(1 * KT + kt) * 512 + ft * 128 + 128],
                            h2T_all[:, kt, j * 256:(j + 1) * 256],
                            start=(fi == 0 and kt == 0),
                            stop=(fi == 1 and kt == KT - 1))
                for fi in range(2):
                    ft = fh * 2 + fi
                    nc.scalar.activation(out=h1T[:, ft, :],
                                         in_=ph[:, fi * 256:(fi + 1) * 256],
                                         func=AF.Gelu, bias=bcol_w(i, 2, ft),
                                         scale=1.0)
            return h1T

        def c2_chunk(i, j, wo, h1T, mv1n):
            """FFN second projection + residual for chunk j; next layer's
            LN1 stats pipelined in when mv1n is given."""
            for st in range(CT):
                pf = psum.tile([128, 512], F32, tag="one", bufs=2)
                for mt in range(KT):
                    nc.tensor.matmul(
                        pf[:, :],
                        h1T[:, mt, st * 128:(st + 1) * 128],
                        wo[:, (2 * KT + mt) * 512:(2 * KT + mt) * 512 + 512],
                        start=(mt == 0),
                        stop=(not with_bias and mt == KT - 1))
                if with_bias:
                    nc.tensor.matmul(pf[:, :], ones_row[:, 0:128],
                                     brow_w(i, 2), start=False, stop=True)
                ls = lat[:, (j * CT + st) * 512:(j * CT + st) * 512 + 512]
                nc.vector.scalar_tensor_tensor(out=ls, in0=pf[:, :], scalar=1.0,
                                               in1=ls, op0=ALU.mult, op1=ALU.add)
                if mv1n is not None:
                    ln_stats(mv1n, j * CT + st, j * CT + st)

        # `repeat` unrolls the FULL computation (conv -> layers -> output
        # projection); every iteration recomputes the same output from x.
        # test.py uses repeat>1 to measure marginal per-iteration device
        # time through the fixed ~80ms launch overhead.
        for _rep in range(repeat):
            # ===== fused input conv (+ LN1 of layer 0 interleaved) =====
            _ph(nc, 'conv')
            wq, wo = load_weights(0)
            x_sb = work.tile([CIN, SH + 2], BF16, tag="x_sb", bufs=2)
            dma(out=x_sb, in_=x_d[:, :])
            mv1 = small.tile([128, 2 * NT8], F32, tag="mv1")
            for ct8 in range(NT8):
                pc = psum.tile([128, 512], F32, tag="one", bufs=2)
                for d in range(3):
                    nc.tensor.matmul(pc[:, :],
                                     x_sb[:, ct8 * 128 + d: ct8 * 128 + d + 128],
                                     cvWT[:, d * 512:(d + 1) * 512],
                                     start=(d == 0),
                                     stop=(not with_bias and d == 2))
                if with_bias:
                    for d in range(3):
                        nc.tensor.matmul(pc[:, :],
                                         halo[:, ct8 * 128 + d: ct8 * 128 + d + 128],
                                         cvb_row(d), start=False, stop=False)
                    nc.tensor.matmul(pc[:, :], ones_row[:, 0:128], ccb_row(),
                                     start=False, stop=True)
                nc.scalar.copy(out=lat[:, ct8 * 512:(ct8 + 1) * 512], in_=pc[:, :])
                ln_stats(mv1, ct8, ct8)
                if ct8 == 3:
                    ln1_sweep(mv1, 0)
                elif ct8 == 7:
                    ln1_sweep(mv1, 2)

            # ===== layer-0 pass 1 =====
            _ph(nc, 'L0_pass1')
            pass1_pair(0, 0, wq)
            pass1_pair(0, 2, wq)

            for i in range(L):
                last = (i + 1 == L)
                _ph(nc, f'L{i}_exch')
                exchange()
                _ph(nc, f'L{i}_pass2a')
                if not last:
                    wq_n, wo_n = load_weights(i + 1)
                mv2, qT_all, smks = pass2a(i, wq)
                _ph(nc, f'L{i}_pass2b')
                intra_seed, prefix_fin, oproj, sweep2 = \
                    make_pass2b(i, wo, mv2, qT_all, smks)
                if not last:
                    mv1n = small.tile([128, 2 * NT8], F32, tag="mv1")
                else:
                    mv1n = None

                # chunk-granular pipeline across pass2b + FFN
                intra_seed(0)
                intra_seed(1)
                prefix_fin(0)
                prefix_fin(1)
                intra_seed(2)
                oproj(0)
                prefix_fin(2)
                intra_seed(3)
                sweep2(0)
                oproj(1)
                prefix_fin(3)
                sweep2(1)
                h1T0 = c1_chunk(i, 0, wo)
                oproj(2)
                sweep2(2)
                h1T1 = c1_chunk(i, 1, wo)
                c2_chunk(i, 0, wo, h1T0, mv1n)
                oproj(3)
                sweep2(3)
                h1T2 = c1_chunk(i, 2, wo)
                c2_chunk(i, 1, wo, h1T1, mv1n)
                if not last:
                    ln1_sweep(mv1n, 0)
                _ph(nc, f'L{i}_pass3')
                c2_chunk(i, 2, wo, h1T2, mv1n)
                h1T3 = c1_chunk(i, 3, wo)
                if last:
                    latT = work.tile([128, KT, SH], BF16, tag="latT", bufs=1)
                    out_sb = work.tile([COUT, SH], F32, tag="out_sb", bufs=1)
                    for ct8 in range(4):
                        latb = work.tile([128, 512], BF16, tag="latb", bufs=2)
                        nc.scalar.copy(out=latb, in_=lat[:, ct8 * 512:(ct8 + 1) * 512])
                        nc.sync.dma_start_transpose(
                            out=latT[:, :, ct8 * 128:(ct8 + 1) * 128], in_=latb)
                c2_chunk(i, 3, wo, h1T3, mv1n)
                if not last:
                    ln1_sweep(mv1n, 2)
                    _ph(nc, f'L{i + 1}_pass1')
                    pass1_pair(i + 1, 0, wq_n)
                    pass1_pair(i + 1, 2, wq_n)
                    wq, wo = wq_n, wo_n
                else:
                    _ph(nc, 'tail')
                    for ct8 in range(4, NT8):
                        latb = work.tile([128, 512], BF16, tag="latb", bufs=2)
                        nc.scalar.copy(out=latb, in_=lat[:, ct8 * 512:(ct8 + 1) * 512])
                        nc.sync.dma_start_transpose(
                            out=latT[:, :, ct8 * 128:(ct8 + 1) * 128], in_=latb)
                    for sb in range(SH // 512):
                        pout = psum.tile([COUT, 512], F32, tag="one", bufs=2)
                        for kt in range(KT):
                            nc.tensor.matmul(pout[:, :],
                                             outWT[:, kt * COUT:(kt + 1) * COUT],
                                             latT[:, kt, sb * 512:(sb + 1) * 512],
                                             start=(kt == 0),
                                             stop=(not with_bias and kt == KT - 1))
                        if with_bias:
                            nc.tensor.matmul(pout[:, :], outb_row(), ones_row,
                                             start=False, stop=True)
                        nc.scalar.copy(out=out_sb[:, sb * 512:(sb + 1) * 512],
                                       in_=pout[:, :])
                    dma(out=out_d[:, :], in_=out_sb)


# ---------------- host side ----------------

_CACHE = threading.local()


def _get_program(with_bias=False, loop_input=False):
    key = f"nc_{with_bias}_{loop_input}"
    if not hasattr(_CACHE, key):
        setattr(_CACHE, key, build_program(with_bias=with_bias,
                                           loop_input=loop_input))
    return getattr(_CACHE, key)


def _needs_bias(inputs):
    f32 = np.float32
    ln1_b = np.asarray(inputs["ln1_b"], f32)
    ln2_b = np.asarray(inputs["ln2_b"], f32)
    vals = [np.asarray(inputs[k], f32) for k in
            ("in_b", "cc_b", "out_b", "bo", "c2_b")]
    bv_eff = np.asarray(inputs["bv"], f32) + np.einsum(
        "loe,le->lo", np.asarray(inputs["Wv"], f32), ln1_b)
    vals.append(bv_eff)
    return any(np.abs(v).max() > 0 for v in vals)


def _prep_shared(inputs):
    f32 = np.float32
    inW = np.asarray(inputs["in_W"], f32)      # [E, CIN]
    in_b = np.asarray(inputs["in_b"], f32)
    ccW = np.asarray(inputs["cc_W"], f32)      # [E, E, 3]
    cc_b = np.asarray(inputs["cc_b"], f32)
    outW = np.asarray(inputs["out_W"], f32)    # [COUT, E]
    out_b = np.asarray(inputs["out_b"], f32)

    # fused conv: V_d = ccW[:,:,d] @ inW  -> [E, CIN]; store V_d^T
    cvWT = np.zeros((CIN, 3 * E), f32)
    for d in range(3):
        cvWT[:, d * E:(d + 1) * E] = (ccW[:, :, d] @ inW).T

    ln1_g = np.asarray(inputs["ln1_g"], f32); ln1_b = np.asarray(inputs["ln1_b"], f32)
    ln2_g = np.asarray(inputs["ln2_g"], f32); ln2_b = np.asarray(inputs["ln2_b"], f32)

    wpack = np.zeros((128, L * 6 * KT * 512), f32)
    brow = np.zeros((1, BROW_N), f32)
    bcol = np.zeros((128, L * 3 * KT), f32)
    for i in range(L):
        biases = {}
        for w, (Wn, bn, g, bb) in enumerate((
                ("Wq", "bq", ln1_g[i], ln1_b[i]),
                ("Wk", "bk", ln1_g[i], ln1_b[i]),
                ("Wv", "bv", ln1_g[i], ln1_b[i]),
                ("Wo", "bo", None, None),
                ("c1_W", "c1_b", ln2_g[i], ln2_b[i]),
                ("c2_W", "c2_b", None, None))):
            W = np.asarray(inputs[Wn], f32)[i]          # [E_out, E_in]
            bias = np.asarray(inputs[bn], f32)[i].copy()
            if g is not None:
                WT = (W * g[None, :]).T                  # fold LN gain
                bias = bias + W @ bb                     # fold LN bias
            else:
                WT = W.T
            for kt in range(KT):
                wpack[:, (i * 6 + w) * KT * 512 + kt * 512:
                      (i * 6 + w) * KT * 512 + kt * 512 + 512] = \
                    WT[kt * 128:(kt + 1) * 128, :]
            biases[w] = bias
        # rows: bv, bo, c2b
        brow[0, (i * 3 + 0) * 512:(i * 3 + 0) * 512 + 512] = biases[2]
        brow[0, (i * 3 + 1) * 512:(i * 3 + 1) * 512 + 512] = biases[3]
        brow[0, (i * 3 + 2) * 512:(i * 3 + 2) * 512 + 512] = biases[5]
        # cols: bq, bk, c1b
        for which, w in ((0, 0), (1, 1), (2, 4)):
            for kt in range(KT):
                bcol[:, (i * 3 + which) * KT + kt] = biases[w][kt * 128:(kt + 1) * 128]

    outWT = np.zeros((128, KT * COUT), f32)
    for kt in range(KT):
        outWT[:, kt * COUT:(kt + 1) * COUT] = outW.T[kt * 128:(kt + 1) * 128, :]

    # conv-bias rows: (W_d @ in_b) masked by halo at use time; then cc_b, out_b
    for d in range(3):
        brow[0, L * 3 * 512 + d * 512: L * 3 * 512 + (d + 1) * 512] = \
            ccW[:, :, d] @ in_b
    brow[0, L * 3 * 512 + 3 * 512: L * 3 * 512 + 4 * 512] = cc_b
    brow[0, L * 3 * 512 + 4 * 512: L * 3 * 512 + 4 * 512 + COUT] = out_b

    tril = np.tril(np.ones((128, 128), f32)).T  # keep t<=s in [t,s] layout

    return {
        "cvWT": cvWT.astype(BF),
        "wpack": wpack.astype(BF),
        "outWT": outWT.astype(BF),
        "brow": brow.astype(BF),
        "bcol": bcol,
        "tril": tril.astype(BF),
        "ones_row": np.ones((1, 512), f32).astype(BF),
        "ones_col_bf": np.ones((128, 1), f32).astype(BF),
    }


def _prep_core_inputs(shared, inputs, b, h):
    f32 = np.float32
    x = np.asarray(inputs["x"], f32)
    s0 = h * SH
    x_sl = np.zeros((CIN, SH + 2), f32)
    lo = max(0, s0 - 2)
    x_sl[:, 2 - (s0 - lo):] = x[b, :, lo:s0 + SH]
    halo = np.ones((1, SH + 2), f32)
    if h == 0:
        halo[0, :2] = 0.0
    mcol = np.zeros((128, 2), f32)
    mcol[:, 0] = float(h)
    mcol[:, 1] = 1.0 - float(h)
    m = dict(shared)
    m["x_sl"] = x_sl.astype(BF)
    m["halo"] = halo.astype(BF)
    m["mcol"] = mcol
    return m


def _run(inputs, loop_input=False, rept=1, **kw):
    nc = _get_program(with_bias=_needs_bias(inputs), loop_input=loop_input)
    shared = _prep_shared(inputs)
    in_maps = []
    for core in range(NCORES):
        b, h = core // 2, core % 2
        m = _prep_core_inputs(shared, inputs, b, h)
        if loop_input:
            m["rept"] = np.full((1, 1), rept, np.int32)
        in_maps.append(m)
    return run_bass_kernel_spmd(nc, in_maps, core_ids=list(range(NCORES)), **kw)


def kernel(**inputs):
    res = _run(inputs)
    out = np.zeros((B, COUT, S), np.float32)
    for core in range(NCORES):
        b, h = core // 2, core % 2
        out[b, :, h * SH:(h + 1) * SH] = res.results[core]["out"]
    return out


def bench(inputs, trace_cores=(0, 1), tmpdir=None):
    """Run with NTFF tracing; returns BassKernelResults with exec_time_ns."""
    return _run(inputs, trace=True, trace_cores=list(trace_cores), tmpdir=tmpdir)


# revision 32
# speedup vs baseline: 1.1650x; 1.0954x over previous
"""Recurrent linear-attention transformer on 8 Trainium2 NeuronCores.

Sharding: 4-way data parallel over batch x 2-way sequence split (halves of
S=2048). Causal linear attention is computed in chunked form; the only
cross-core dependency is the cumulative (k^T v, sum k) state at the half
boundary, exchanged once per layer via a pairwise AllReduce.

Schedule (v2, PE-density focused):
  - The input projection is folded into the causal conv on the host
    (V_d = ccW[:,:,d] @ in_W), so the conv contracts over CIN=64 only:
    3 matmuls per seq tile instead of 15.
  - LN1 for layer 0 is interleaved with the conv tiles.
  - Per layer: pass1 (v then k projections + per-chunk prefix states) ->
    boundary AllReduce -> pass2a (q projection + masked intra scores,
    overlapping the collective) -> Peff = P_local + R adds on DVE ->
    a chunk-granular pipeline over pass2b + FFN: each chunk's intra
    numerator/denominator matmuls SEED the PSUM accumulation groups
    before R arrives, the prefix matmuls accumulate on top (no identity
    re-injection), and o-projection / LN2 sweep / FFN c1 (N=256 per
    chunk) / c2 are interleaved so the attn-transpose and LN chains hide
    under PE work across the layer boundary into the next pass1.
  - Weight DMAs for layer i+1 are emitted at the top of pass2a(i): DMA
    queues are round-robin in emission order with head-of-line blocking,
    so late-emitted weight loads would queue behind dependency-stalled
    transposes.
  - LN scales use sqrt(var+eps) on ACT + reciprocal on DVE; normalize
    runs on ACT (Identity with per-partition scale/bias) to keep the
    DVE queue shallow. PSUM: tag "one" x3, "big2" (1024) x2, "sm" x1
    = 8 banks; one start per PSUM zero-region per accumulation group.

Per-core layout conventions (SBUF 2-D tensors, 128 partitions):
  lat   f32  [128, 8*512]    seq-major residual: c-tile ct8 block, cols = e
  hT    bf16 [128, 4*4*256]  feature-major LN1 output: (chunk, kt) blocks
  kT    bf16 [128, 4*4*256]  phi(k) feature-major: (chunk, ft) blocks, cols=t
  v     bf16 [128, 4*2*512]  v seq-major: (chunk, tt) blocks, cols = e'
  P_all bf16 [128, 5*4*513]  prefix states: (j, kt) -> [KV[e,e'] | s_k[e]]
All matmul operands bf16, PSUM/stats/residual f32.

`build_program(repeat=R)` unrolls the whole computation R times (identical
output each iteration); test.py uses it to measure per-iteration device time
through the ~80 ms fixed launch overhead of the axon-tunneled PJRT path.
"""

import threading

import numpy as np
import ml_dtypes

import concourse.bass as bass
import concourse.bacc as bacc
import concourse.tile as tile
import concourse.mybir as mybir
from concourse.bass_utils import run_bass_kernel_spmd

AF = mybir.ActivationFunctionType
ALU = mybir.AluOpType
F32 = mybir.dt.float32
BF16 = mybir.dt.bfloat16
BF = ml_dtypes.bfloat16

L, B, CIN, COUT, E, S = 4, 4, 64, 64, 512, 2048
NCORES = 8
SH = S // 2          # per-core sequence half
C = 256              # attention chunk
NCH = SH // C        # 4 chunks
CT = C // 128        # 2 c-tiles per chunk
KT = E // 128        # 4 feature tiles
NT8 = SH // 128      # 8 seq tiles per half
EPS = 1e-6
LN_EPS = 1e-5
SKW = E + 1          # 513: KV block plus s_k column

REPLICA_GROUPS = [[0, 1], [2, 3], [4, 5], [6, 7]]

PHASE_HOOK = None  # optional (name, next_instr_id) recorder for profiling


def _ph(nc, name):
    if PHASE_HOOK is not None:
        PHASE_HOOK(name, nc.next_id())

# brow layout (bf16): per-layer [bv, bo, c2b] rows, then 3 conv-bias rows
# (W_d @ in_b), cc_b, out_b
BROW_N = L * 3 * 512 + 3 * 512 + 512 + COUT


def build_program(cc=True, repeat=1, with_bias=False, loop_input=False, no_r=False):
    nc = bacc.Bacc("TRN2", target_bir_lowering=False, debug=False,
                   num_devices=NCORES)

    rept_d = (nc.dram_tensor("rept", [1, 1], mybir.dt.int32,
                             kind="ExternalInput") if loop_input else None)
    x_d = nc.dram_tensor("x_sl", [CIN, SH + 2], BF16, kind="ExternalInput")
    cvWT_d = nc.dram_tensor("cvWT", [CIN, 3 * E], BF16, kind="ExternalInput")
    wpack_d = nc.dram_tensor("wpack", [128, L * 6 * KT * 512], BF16, kind="ExternalInput")
    outWT_d = nc.dram_tensor("outWT", [128, KT * COUT], BF16, kind="ExternalInput")
    brow_d = nc.dram_tensor("brow", [1, BROW_N], BF16, kind="ExternalInput")
    bcol_d = nc.dram_tensor("bcol", [128, L * 3 * KT], F32, kind="ExternalInput")
    tril_d = nc.dram_tensor("tril", [128, 128], BF16, kind="ExternalInput")
    mcol_d = nc.dram_tensor("mcol", [128, 2], F32, kind="ExternalInput")  # [m, 1-m]
    halo_d = nc.dram_tensor("halo", [1, SH + 2], BF16, kind="ExternalInput")
    ones_row_d = nc.dram_tensor("ones_row", [1, 512], BF16, kind="ExternalInput")
    ones_col_bf_d = nc.dram_tensor("ones_col_bf", [128, 1], BF16, kind="ExternalInput")

    out_d = nc.dram_tensor("out", [COUT, SH], F32, kind="ExternalOutput")

    with tile.TileContext(nc, num_cores=NCORES) as tc:
        _emit(nc, tc, x_d, cvWT_d, wpack_d, outWT_d, brow_d, bcol_d,
              tril_d, mcol_d, halo_d, ones_row_d,
              ones_col_bf_d, out_d, cc=cc, repeat=repeat,
              with_bias=with_bias, rept_d=rept_d, no_r=no_r)
    nc.compile()
    return nc


def _emit(nc, tc, x_d, cvWT_d, wpack_d, outWT_d, brow_d, bcol_d,
          tril_d, mcol_d, halo_d, ones_row_d,
          ones_col_bf_d, out_d, cc=True, repeat=1,
          with_bias=False, rept_d=None, no_r=False):
    import contextlib
    ctx = contextlib.ExitStack()
    with ctx:
        singles = ctx.enter_context(tc.tile_pool(name="singles", bufs=1))
        persist = ctx.enter_context(tc.tile_pool(name="persist", bufs=1))
        small = ctx.enter_context(tc.tile_pool(name="small", bufs=4))
        psum = ctx.enter_context(tc.tile_pool(name="psum", bufs=2, space="PSUM"))
        dram = ctx.enter_context(tc.tile_pool(name="dram", bufs=2, space="DRAM"))

        dma = nc.sync.dma_start

        # ---- constants ----
        outWT = singles.tile([128, KT * COUT], BF16)
        dma(out=outWT, in_=outWT_d[:, :])
        cvWT = singles.tile([CIN, 3 * E], BF16)
        dma(out=cvWT, in_=cvWT_d[:, :])
        bcol = singles.tile([128, L * 3 * KT], F32)
        dma(out=bcol, in_=bcol_d[:, :])
        tril = singles.tile([128, 128], BF16)
        dma(out=tril, in_=tril_d[:, :])
        mcol = singles.tile([128, 2], F32)
        dma(out=mcol, in_=mcol_d[:, :])
        ones_col_bf = singles.tile([128, 1], BF16)
        dma(out=ones_col_bf, in_=ones_col_bf_d[:, :])
        eps_col = singles.tile([128, 1], F32)
        nc.vector.memset(eps_col, LN_EPS)
        if with_bias:
            brow = singles.tile([1, BROW_N], BF16)
            dma(out=brow, in_=brow_d[:, :])
            halo = singles.tile([1, SH + 2], BF16)
            dma(out=halo, in_=halo_d[:, :])
            ones_row = singles.tile([1, 512], BF16)
            dma(out=ones_row, in_=ones_row_d[:, :])

        cvb_row = lambda d: brow[:, L * 3 * 512 + d * 512: L * 3 * 512 + (d + 1) * 512]
        ccb_row = lambda: brow[:, L * 3 * 512 + 3 * 512: L * 3 * 512 + 4 * 512]
        outb_row = lambda: brow[:, L * 3 * 512 + 4 * 512: L * 3 * 512 + 4 * 512 + COUT]

        def brow_w(i, w):
            # w: 0=bv 1=bo 2=c2b
            return brow[:, (i * 3 + w) * 512:(i * 3 + w) * 512 + 512]

        def bcol_w(i, which, kt):
            # which: 0=bq 1=bk 2=c1b
            c = (i * 3 + which) * KT + kt
            return bcol[:, c:c + 1]

        # ---- persistent state ----
        lat = persist.tile([128, NT8 * 512], F32)
        P_all = persist.tile([128, (NCH + 1) * KT * SKW], BF16)
        nc.vector.memset(P_all[:, 0:KT * SKW], 0.0)
        # feature-tile-major activations: [within-tile row, tile, seq col]
        hT_all = persist.tile([128, KT, SH // 4 * 4], BF16)   # [e%128, kt, s]
        kT_all = persist.tile([128, KT, SH // 4 * 4], BF16)   # [feat%128, ft, t]
        h2T_all = persist.tile([128, KT, SH // 4 * 4], BF16)
        v_all = persist.tile([128, NCH * CT * 512], BF16)
        R_sb = persist.tile([128, KT * SKW], BF16)

        def Pb(j, kt):
            o = (j * KT + kt) * SKW
            return P_all[:, o:o + SKW]

        # layer-loop pools (persist across repeats)
        wbufs = 1 if with_bias else 2
        wq_pool = ctx.enter_context(tc.tile_pool(name="wq_pool", bufs=wbufs))
        wo_pool = ctx.enter_context(tc.tile_pool(name="wo_pool", bufs=wbufs))
        work = ctx.enter_context(tc.tile_pool(name="work", bufs=2))

        def load_weights(i):
            """DMA layer-i weights; returns (wq, wo) tiles."""
            wq = wq_pool.tile([128, 3 * KT * 512], BF16, tag="wq")
            wo = wo_pool.tile([128, 3 * KT * 512], BF16, tag="wo")
            for wti in range(3):
                dma(out=wq[:, wti * KT * 512:(wti + 1) * KT * 512],
                    in_=wpack_d[:, (i * 6 + wti) * KT * 512:(i * 6 + wti + 1) * KT * 512])
                dma(out=wo[:, wti * KT * 512:(wti + 1) * KT * 512],
                    in_=wpack_d[:, (i * 6 + 3 + wti) * KT * 512:(i * 6 + 4 + wti) * KT * 512])
            return wq, wo

        # ---------- layernorm helpers ----------
        def ln_stats(mv8, ct8, sl):
            """bn stats of lat c-tile ct8 into mv8[:, 2*sl:2*sl+2]."""
            stats = small.tile([128, 6], F32, tag="lnst")
            nc.vector.bn_stats(out=stats, in_=lat[:, ct8 * 512:(ct8 + 1) * 512])
            nc.vector.bn_aggr(out=mv8[:, 2 * sl:2 * sl + 2], in_=stats)

        def ln_scales(mv8, n, tag):
            """From interleaved [mean,var] pairs build rstd [128,n] and
            nb = -mean*rstd [128,n] (ACT sqrt + DVE reciprocal)."""
            sd = small.tile([128, n], F32, tag=tag + "sd")
            rstd = small.tile([128, n], F32, tag=tag + "rs")
            nb = small.tile([128, n], F32, tag=tag + "nb")
            mvv = mv8[:, 0:2 * n].rearrange("p (n two) -> p n two", two=2)
            nc.scalar.activation(out=sd, in_=mvv[:, :, 1], func=AF.Sqrt,
                                 bias=eps_col[:, 0:1], scale=1.0)
            nc.vector.reciprocal(out=rstd, in_=sd)
            nc.vector.scalar_tensor_tensor(out=nb, in0=mvv[:, :, 0], scalar=-1.0,
                                           in1=rstd, op0=ALU.mult, op1=ALU.mult)
            return nb, rstd

        def ln_norm(dst_y, ct8, nb, rstd, sl):
            nc.scalar.activation(out=dst_y, in_=lat[:, ct8 * 512:(ct8 + 1) * 512],
                                 func=AF.Identity, bias=nb[:, sl:sl + 1],
                                 scale=rstd[:, sl:sl + 1])

        def ln1_sweep(mv, jp):
            """Per-pair LN1 finish: rsqrt batch, normalize, transpose to hT."""
            nb, rstd = ln_scales(mv[:, jp * CT * 2: jp * CT * 2 + 8], 4, "l1")
            for c4 in range(4):
                y = work.tile([128, 512], BF16, tag="y", bufs=2)
                ln_norm(y, jp * CT + c4, nb, rstd, c4)
                nc.sync.dma_start_transpose(
                    out=hT_all[:, :, jp * 256 + c4 * 128: jp * 256 + c4 * 128 + 128],
                    in_=y)

        def pair_proj_phi(wt, w, jp, dst3, i, which):
            """Feature-major projection for a chunk pair with phi applied.
            dst3: 3D [128, KT, SH] tile written at cols [jp*256, jp*256+512).
            PSUM comes from the deep 'one' pool so the big2 rotation stays
            exclusive to the attention-numerator / FFN-c1 pipeline."""
            for fh in range(2):
                pps = []
                for fi in range(2):
                    pp = psum.tile([128, 512], F32, tag="one", bufs=3)
                    pps.append(pp)
                    ft = fh * 2 + fi
                    for kt in range(KT):
                        nc.tensor.matmul(
                            pp[:, :],
                            wt[:, (w * KT + kt) * 512 + ft * 128:(w * KT + kt) * 512 + ft * 128 + 128],
                            hT_all[:, kt, jp * 256: jp * 256 + 512],
                            start=(kt == 0), stop=(kt == KT - 1))
                et = work.tile([128, 1024], BF16, tag="phiE")
                for fi in range(2):
                    ft = fh * 2 + fi
                    bc = bcol_w(i, which, ft)
                    nc.scalar.activation(out=et[:, fi * 512:(fi + 1) * 512],
                                         in_=pps[fi][:, :],
                                         func=AF.Exp, bias=bc, scale=1.0)
                    nc.scalar.activation(out=dst3[:, ft, jp * 256: jp * 256 + 512],
                                         in_=pps[fi][:, :],
                                         func=AF.Relu, bias=bc, scale=1.0)
                d = dst3[:, fh * 2: fh * 2 + 2, jp * 256: jp * 256 + 512]
                nc.vector.scalar_tensor_tensor(out=d, in0=et, scalar=1.0, in1=d,
                                               op0=ALU.min, op1=ALU.add)

        def pass1_pair(i, jp, wq):
            """k/v projections + prefix-state chain for chunks jp, jp+1.
            v matmuls are emitted before the k projection so their PSUM
            drains are not queued behind the phi activations."""
            for j in (jp, jp + 1):
                for tt in range(CT):
                    pv = psum.tile([128, 512], F32, tag="one", bufs=2)
                    for kt in range(KT):
                        nc.tensor.matmul(
                            pv[:, :],
                            hT_all[:, kt, j * 256 + tt * 128: j * 256 + tt * 128 + 128],
                            wq[:, (2 * KT + kt) * 512:(2 * KT + kt) * 512 + 512],
                            start=(kt == 0),
                            stop=(not with_bias and kt == KT - 1))
                    if with_bias:
                        nc.tensor.matmul(pv[:, :], ones_row[:, 0:128],
                                         brow_w(i, 0), start=False, stop=True)
                    nc.scalar.copy(
                        out=v_all[:, (j * CT + tt) * 512:(j * CT + tt) * 512 + 512],
                        in_=pv[:, :])

            pair_proj_phi(wq, 1, jp, kT_all, i, 1)

            kseqs = {}
            for j in (jp, jp + 1):
                # kseq[t%128, tt, e] = phi(k)[e, t] transposed
                kseq = work.tile([128, CT, 512], BF16, tag=f"kseq{j % 2}",
                                 bufs=1)
                kseqs[j] = kseq
                for ft in range(KT):
                    nc.sync.dma_start_transpose(
                        out=kseq[:, :, ft * 128:(ft + 1) * 128],
                        in_=kT_all[:, ft, j * 256:(j + 1) * 256])

            # delta state + prefix chain:  P[j+1] = P[j] + kseq^T [v|1]
            for j in (jp, jp + 1):
                kseq = kseqs[j]
                skp = psum.tile([128, KT], F32, tag="sm", bufs=2)
                for kt in range(KT):
                    pd = psum.tile([128, 512], F32, tag="one", bufs=2)
                    for tt in range(CT):
                        ks = kseq[:, tt, kt * 128:(kt + 1) * 128]
                        nc.tensor.matmul(
                            pd[:, :], ks,
                            v_all[:, (j * CT + tt) * 512:(j * CT + tt) * 512 + 512],
                            start=(tt == 0), stop=(tt == CT - 1))
                        nc.tensor.matmul(
                            skp[:, kt:kt + 1], ks, ones_col_bf,
                            start=(kt == 0 and tt == 0),
                            stop=(kt == KT - 1 and tt == CT - 1))
                    nc.vector.scalar_tensor_tensor(
                        out=Pb(j + 1, kt)[:, 0:E], in0=pd[:, :], scalar=1.0,
                        in1=Pb(j, kt)[:, 0:E], op0=ALU.mult, op1=ALU.add)
                for kt in range(KT):
                    nc.vector.scalar_tensor_tensor(
                        out=Pb(j + 1, kt)[:, E:SKW], in0=skp[:, kt:kt + 1], scalar=1.0,
                        in1=Pb(j, kt)[:, E:SKW], op0=ALU.mult, op1=ALU.add)

        def exchange():
            """Boundary-state AllReduce (pairwise)."""
            contrib = work.tile([128, KT * SKW], BF16, tag="contrib", bufs=1)
            nc.vector.tensor_scalar_mul(contrib,
                                        P_all[:, NCH * KT * SKW:(NCH + 1) * KT * SKW],
                                        mcol[:, 1:2])
            cc_out = dram.tile([128, KT * SKW], BF16, tag="cc_out")
            cc_in = dram.tile([128, KT * SKW], BF16, tag="cc_in")
            dma(out=cc_out, in_=contrib)
            if cc:
                nc.gpsimd.collective_compute(
                    "AllReduce", ALU.add, replica_groups=REPLICA_GROUPS,
                    ins=[cc_out.opt()], outs=[cc_in.opt()])
            else:
                nc.gpsimd.dma_start(out=cc_in.opt(), in_=cc_out.opt())
            dma(out=R_sb, in_=cc_in)

        def pass2a(i, wq):
            """q projection + masked intra-chunk scores (R-independent,
            overlaps the collective). The intra numerator/denominator
            matmuls are deferred to pass2b where they share the prefix
            PSUM accumulation groups."""
            mv2 = small.tile([128, 2 * NT8], F32, tag="mv2")
            qT_all = work.tile([128, KT, SH], BF16, tag="qTall", bufs=1)
            smks = {}
            for jp in range(0, NCH, 2):
                pair_proj_phi(wq, 0, jp, qT_all, i, 0)

                for j in (jp, jp + 1):
                    jo = j * 256
                    # scoresT: cols 0:256 = t0 x (s0|s1); cols 256:384 = t1 x s1
                    ps = psum.tile([128, 384], F32, tag="one", bufs=2)
                    for ft in range(KT):
                        nc.tensor.matmul(
                            ps[:, 0:256],
                            kT_all[:, ft, j * 256: j * 256 + 128],
                            qT_all[:, ft, jo: jo + 256],
                            start=(ft == 0), stop=False)
                        nc.tensor.matmul(
                            ps[:, 256:384],
                            kT_all[:, ft, j * 256 + 128: j * 256 + 256],
                            qT_all[:, ft, jo + 128: jo + 256],
                            start=False, stop=(ft == KT - 1))
                    sm = work.tile([128, 384], BF16, tag="smk", bufs=4)
                    smks[j] = sm
                    nc.vector.tensor_mul(sm[:, 0:128], ps[:, 0:128], tril)
                    nc.vector.tensor_copy(out=sm[:, 128:256], in_=ps[:, 128:256])
                    nc.vector.tensor_mul(sm[:, 256:384], ps[:, 256:384], tril)
            return mv2, qT_all, smks

        def make_pass2b(i, wo, mv2, qT_all, smks):
            """Helpers for prefix attention, chunk-granular."""
            if not no_r:
                nc.vector.tensor_scalar_mul(R_sb, R_sb, mcol[:, 0:1])
            Rloc = P_all[:, 0:KT * SKW] if no_r else R_sb
            peffs = {}
            for j in (1, 2, 3):
                peff = work.tile([128, KT * SKW], BF16, tag="peff", bufs=3)
                peffs[j] = peff
                nc.vector.tensor_tensor(
                    out=peff,
                    in0=P_all[:, j * KT * SKW:(j + 1) * KT * SKW],
                    in1=Rloc, op=ALU.add)

            state = {}

            def intra_seed(j):
                """R-independent seeds: intra numerator + denominator
                matmuls open the PSUM accumulation groups."""
                sm = smks[j]
                v0 = v_all[:, (j * CT + 0) * 512:(j * CT + 0) * 512 + 512]
                v1 = v_all[:, (j * CT + 1) * 512:(j * CT + 1) * 512 + 512]
                pn = psum.tile([128, 1024], F32, tag="big2", bufs=2)
                nc.tensor.matmul(pn[:, 0:512], sm[:, 0:128], v0,
                                 start=True, stop=False)
                nc.tensor.matmul(pn[:, 512:1024], sm[:, 128:256], v0,
                                 start=True, stop=False)
                nc.tensor.matmul(pn[:, 512:1024], sm[:, 256:384], v1,
                                 start=False, stop=False)
                pden = psum.tile([128, CT], F32, tag="sm", bufs=2)
                nc.tensor.matmul(pden[:, 0:1], sm[:, 0:128], ones_col_bf,
                                 start=True, stop=False)
                nc.tensor.matmul(pden[:, 1:2], sm[:, 128:256], ones_col_bf,
                                 start=False, stop=False)
                nc.tensor.matmul(pden[:, 1:2], sm[:, 256:384], ones_col_bf,
                                 start=False, stop=False)
                state[j] = (pn, pden)

            def prefix_fin(j):
                jo = j * 256
                pn, pden = state.pop(j)
                Peff = Rloc if j == 0 else peffs[j]
                for kt in range(KT):
                    nc.tensor.matmul(pn[:, 0:512],
                                     qT_all[:, kt, jo: jo + 128],
                                     Peff[:, kt * SKW: kt * SKW + E],
                                     start=False, stop=(kt == KT - 1))
                for kt in range(KT):
                    nc.tensor.matmul(pn[:, 512:1024],
                                     qT_all[:, kt, jo + 128: jo + 256],
                                     Peff[:, kt * SKW: kt * SKW + E],
                                     start=False, stop=(kt == KT - 1))
                for st in range(CT):
                    for kt in range(KT):
                        nc.tensor.matmul(
                            pden[:, st:st + 1],
                            qT_all[:, kt, jo + st * 128: jo + st * 128 + 128],
                            Peff[:, kt * SKW + E: kt * SKW + SKW],
                            start=False,
                            stop=(st == CT - 1 and kt == KT - 1))
                den = small.tile([128, CT], F32, tag="den")
                nc.vector.tensor_scalar_add(den, pden[:, :], EPS)
                rden = small.tile([128, CT], F32, tag="rden")
                nc.vector.reciprocal(out=rden, in_=den)

                # Unscaled num goes straight to the o-projection; 1/den is
                # applied at the residual (it commutes per-s). With biases
                # the scale must happen before adding b_o: scale here.
                attn = work.tile([128, CT * 512], BF16, tag="attnA", bufs=2)
                sc = (rden[:, 0:1], rden[:, 1:2]) if with_bias else (1.0, 1.0)
                nc.scalar.activation(out=attn[:, 0:512], in_=pn[:, 0:512],
                                     func=AF.Copy, scale=sc[0])
                nc.scalar.activation(out=attn[:, 512:1024], in_=pn[:, 512:1024],
                                     func=AF.Copy, scale=sc[1])
                attnT = work.tile([128, KT, 256], BF16, tag="attnT", bufs=2)
                for st in range(CT):
                    nc.sync.dma_start_transpose(
                        out=attnT[:, :, st * 128:(st + 1) * 128],
                        in_=attn[:, st * 512:(st + 1) * 512])
                state[j] = (attnT, rden)

            def oproj(j):
                attnT, rden = state.pop(j)
                for st in range(CT):
                    po = psum.tile([128, 512], F32, tag="one", bufs=2)
                    for mt in range(KT):
                        nc.tensor.matmul(po[:, :],
                                         attnT[:, mt, st * 128:(st + 1) * 128],
                                         wo[:, (0 * KT + mt) * 512:(0 * KT + mt) * 512 + 512],
                                         start=(mt == 0),
                                         stop=(not with_bias and mt == KT - 1))
                    if with_bias:
                        nc.tensor.matmul(po[:, :], ones_row[:, 0:128],
                                         brow_w(i, 1), start=False, stop=True)
                    ls = lat[:, (j * CT + st) * 512:(j * CT + st) * 512 + 512]
                    nc.vector.scalar_tensor_tensor(out=ls, in0=po[:, :],
                                                   scalar=(1.0 if with_bias
                                                           else rden[:, st:st + 1]),
                                                   in1=ls, op0=ALU.mult, op1=ALU.add)
                    ln_stats(mv2, j * CT + st, j * CT + st)

            def sweep2(j):
                """LN2 normalize + transpose for chunk j's two c-tiles."""
                nb2, rstd2 = ln_scales(mv2[:, j * CT * 2: j * CT * 2 + 4],
                                       2, "l2")
                for c2_ in range(CT):
                    ct8 = j * CT + c2_
                    y2 = work.tile([128, 512], BF16, tag="y2", bufs=2)
                    ln_norm(y2, ct8, nb2, rstd2, c2_)
                    nc.sync.dma_start_transpose(
                        out=h2T_all[:, :, ct8 * 128:(ct8 + 1) * 128],
                        in_=y2)

            return intra_seed, prefix_fin, oproj, sweep2

        def c1_chunk(i, j, wo):
            """FFN first projection + gelu for chunk j (rhs N=256)."""
            h1T = work.tile([128, KT, 256], BF16, tag="h1T", bufs=2)
            ph = psum.tile([128, 1024], F32, tag="big2", bufs=2)
            for ft in range(KT):
                for kt in range(KT):
                    nc.tensor.matmul(
                        ph[:, ft * 256:(ft + 1) * 256],
                        wo[:, (1 * KT + kt) * 512 + ft * 128:(1 * KT + kt) * 512 + ft * 128 + 128],
                        h2T_all[:, kt, j * 256:(j + 1) * 256],
                        start=(kt == 0), stop=(kt == KT - 1))
            for ft in range(KT):
                nc.scalar.activation(out=h1T[:, ft, :],
                                     in_=ph[:, ft * 256:(ft + 1) * 256],
                                     func=AF.Gelu, bias=bcol_w(i, 2, ft),
                                     scale=1.0)
            return h1T

        def c2_chunk(i, j, wo, h1T, mv1n):
            """FFN second projection + residual for chunk j; next layer's
            LN1 stats pipelined in when mv1n is given."""
            for st in range(CT):
                pf = psum.tile([128, 512], F32, tag="one", bufs=2)
                for mt in range(KT):
                    nc.tensor.matmul(
                        pf[:, :],
                        h1T[:, mt, st * 128:(st + 1) * 128],
                        wo[:, (2 * KT + mt) * 512:(2 * KT + mt) * 512 + 512],
                        start=(mt == 0),
                        stop=(not with_bias and mt == KT - 1))
                if with_bias:
                    nc.tensor.matmul(pf[:, :], ones_row[:, 0:128],
                                     brow_w(i, 2), start=False, stop=True)
                ls = lat[:, (j * CT + st) * 512:(j * CT + st) * 512 + 512]
                nc.vector.scalar_tensor_tensor(out=ls, in0=pf[:, :], scalar=1.0,
                                               in1=ls, op0=ALU.mult, op1=ALU.add)
                if mv1n is not None:
                    ln_stats(mv1n, j * CT + st, j * CT + st)

        # `repeat` unrolls the FULL computation (conv -> layers -> output
        # projection); every iteration recomputes the same output from x.
        # test.py uses repeat>1 to measure marginal per-iteration device
        # time through the fixed ~80ms launch overhead.
        for _rep in range(repeat):
            # ===== fused input conv (+ LN1 of layer 0 interleaved) =====
            _ph(nc, 'conv')
            wq, wo = load_weights(0)
            x_sb = work.tile([CIN, SH + 2], BF16, tag="x_sb", bufs=2)
            dma(out=x_sb, in_=x_d[:, :])
            mv1 = small.tile([128, 2 * NT8], F32, tag="mv1")
            for ct8 in range(NT8):
                pc = psum.tile([128, 512], F32, tag="one", bufs=2)
                for d in range(3):
                    nc.tensor.matmul(pc[:, :],
                                     x_sb[:, ct8 * 128 + d: ct8 * 128 + d + 128],
                                     cvWT[:, d * 512:(d + 1) * 512],
                                     start=(d == 0),
                                     stop=(not with_bias and d == 2))
                if with_bias:
                    for d in range(3):
                        nc.tensor.matmul(pc[:, :],
                                         halo[:, ct8 * 128 + d: ct8 * 128 + d + 128],
                                         cvb_row(d), start=False, stop=False)
                    nc.tensor.matmul(pc[:, :], ones_row[:, 0:128], ccb_row(),
                                     start=False, stop=True)
                nc.scalar.copy(out=lat[:, ct8 * 512:(ct8 + 1) * 512], in_=pc[:, :])
                ln_stats(mv1, ct8, ct8)
                if ct8 == 3:
                    ln1_sweep(mv1, 0)
                elif ct8 == 7:
                    ln1_sweep(mv1, 2)

            # ===== layer-0 pass 1 =====
            _ph(nc, 'L0_pass1')
            pass1_pair(0, 0, wq)
            pass1_pair(0, 2, wq)

            for i in range(L):
                last = (i + 1 == L)
                _ph(nc, f'L{i}_exch')
                exchange()
                _ph(nc, f'L{i}_pass2a')
                if not last:
                    wq_n, wo_n = load_weights(i + 1)
                mv2, qT_all, smks = pass2a(i, wq)
                _ph(nc, f'L{i}_pass2b')
                intra_seed, prefix_fin, oproj, sweep2 = \
                    make_pass2b(i, wo, mv2, qT_all, smks)
                if not last:
                    mv1n = small.tile([128, 2 * NT8], F32, tag="mv1")
                else:
                    mv1n = None

                # chunk-granular pipeline across pass2b + FFN
                intra_seed(0)
                intra_seed(1)
                prefix_fin(0)
                prefix_fin(1)
                intra_seed(2)
                oproj(0)
                prefix_fin(2)
                intra_seed(3)
                sweep2(0)
                oproj(1)
                prefix_fin(3)
                sweep2(1)
                h1T0 = c1_chunk(i, 0, wo)
                oproj(2)
                sweep2(2)
                h1T1 = c1_chunk(i, 1, wo)
                c2_chunk(i, 0, wo, h1T0, mv1n)
                oproj(3)
                sweep2(3)
                h1T2 = c1_chunk(i, 2, wo)
                c2_chunk(i, 1, wo, h1T1, mv1n)
                if not last:
                    ln1_sweep(mv1n, 0)
                _ph(nc, f'L{i}_pass3')
                c2_chunk(i, 2, wo, h1T2, mv1n)
                h1T3 = c1_chunk(i, 3, wo)
                if last:
                    latT = work.tile([128, KT, SH], BF16, tag="latT", bufs=1)
                    out_sb = work.tile([COUT, SH], F32, tag="out_sb", bufs=1)
                    for ct8 in range(4):
                        latb = work.tile([128, 512], BF16, tag="latb", bufs=2)
                        nc.scalar.copy(out=latb, in_=lat[:, ct8 * 512:(ct8 + 1) * 512])
                        nc.sync.dma_start_transpose(
                            out=latT[:, :, ct8 * 128:(ct8 + 1) * 128], in_=latb)
                c2_chunk(i, 3, wo, h1T3, mv1n)
                if not last:
                    ln1_sweep(mv1n, 2)
                    _ph(nc, f'L{i + 1}_pass1')
                    pass1_pair(i + 1, 0, wq_n)
                    pass1_pair(i + 1, 2, wq_n)
                    wq, wo = wq_n, wo_n
                else:
                    _ph(nc, 'tail')
                    for ct8 in range(4, NT8):
                        latb = work.tile([128, 512], BF16, tag="latb", bufs=2)
                        nc.scalar.copy(out=latb, in_=lat[:, ct8 * 512:(ct8 + 1) * 512])
                        nc.sync.dma_start_transpose(
                            out=latT[:, :, ct8 * 128:(ct8 + 1) * 128], in_=latb)
                    for sb in range(SH // 512):
                        pout = psum.tile([COUT, 512], F32, tag="one", bufs=2)
                        for kt in range(KT):
                            nc.tensor.matmul(pout[:, :],
                                             outWT[:, kt * COUT:(kt + 1) * COUT],
                                             latT[:, kt, sb * 512:(sb + 1) * 512],
                                             start=(kt == 0),
                                             stop=(not with_bias and kt == KT - 1))
                        if with_bias:
                            nc.tensor.matmul(pout[:, :], outb_row(), ones_row,
                                             start=False, stop=True)
                        nc.scalar.copy(out=out_sb[:, sb * 512:(sb + 1) * 512],
                                       in_=pout[:, :])
                    dma(out=out_d[:, :], in_=out_sb)


# ---------------- host side ----------------

_CACHE = threading.local()


def _get_program(with_bias=False, loop_input=False):
    key = f"nc_{with_bias}_{loop_input}"
    if not hasattr(_CACHE, key):
        setattr(_CACHE, key, build_program(with_bias=with_bias,
                                           loop_input=loop_input))
    return getattr(_CACHE, key)


def _needs_bias(inputs):
    f32 = np.float32
    ln1_b = np.asarray(inputs["ln1_b"], f32)
    ln2_b = np.asarray(inputs["ln2_b"], f32)
    vals = [np.asarray(inputs[k], f32) for k in
            ("in_b", "cc_b", "out_b", "bo", "c2_b")]
    bv_eff = np.asarray(inputs["bv"], f32) + np.einsum(
        "loe,le->lo", np.asarray(inputs["Wv"], f32), ln1_b)
    vals.append(bv_eff)
    return any(np.abs(v).max() > 0 for v in vals)


def _prep_shared(inputs):
    f32 = np.float32
    inW = np.asarray(inputs["in_W"], f32)      # [E, CIN]
    in_b = np.asarray(inputs["in_b"], f32)
    ccW = np.asarray(inputs["cc_W"], f32)      # [E, E, 3]
    cc_b = np.asarray(inputs["cc_b"], f32)
    outW = np.asarray(inputs["out_W"], f32)    # [COUT, E]
    out_b = np.asarray(inputs["out_b"], f32)

    # fused conv: V_d = ccW[:,:,d] @ inW  -> [E, CIN]; store V_d^T
    cvWT = np.zeros((CIN, 3 * E), f32)
    for d in range(3):
        cvWT[:, d * E:(d + 1) * E] = (ccW[:, :, d] @ inW).T

    ln1_g = np.asarray(inputs["ln1_g"], f32); ln1_b = np.asarray(inputs["ln1_b"], f32)
    ln2_g = np.asarray(inputs["ln2_g"], f32); ln2_b = np.asarray(inputs["ln2_b"], f32)

    wpack = np.zeros((128, L * 6 * KT * 512), f32)
    brow = np.zeros((1, BROW_N), f32)
    bcol = np.zeros((128, L * 3 * KT), f32)
    for i in range(L):
        biases = {}
        for w, (Wn, bn, g, bb) in enumerate((
                ("Wq", "bq", ln1_g[i], ln1_b[i]),
                ("Wk", "bk", ln1_g[i], ln1_b[i]),
                ("Wv", "bv", ln1_g[i], ln1_b[i]),
                ("Wo", "bo", None, None),
                ("c1_W", "c1_b", ln2_g[i], ln2_b[i]),
                ("c2_W", "c2_b", None, None))):
            W = np.asarray(inputs[Wn], f32)[i]          # [E_out, E_in]
            bias = np.asarray(inputs[bn], f32)[i].copy()
            if g is not None:
                WT = (W * g[None, :]).T                  # fold LN gain
                bias = bias + W @ bb                     # fold LN bias
            else:
                WT = W.T
            for kt in range(KT):
                wpack[:, (i * 6 + w) * KT * 512 + kt * 512:
                      (i * 6 + w) * KT * 512 + kt * 512 + 512] = \
                    WT[kt * 128:(kt + 1) * 128, :]
            biases[w] = bias
        # rows: bv, bo, c2b
        brow[0, (i * 3 + 0) * 512:(i * 3 + 0) * 512 + 512] = biases[2]
        brow[0, (i * 3 + 1) * 512:(i * 3 + 1) * 512 + 512] = biases[3]
        brow[0, (i * 3 + 2) * 512:(i * 3 + 2) * 512 + 512] = biases[5]
        # cols: bq, bk, c1b
        for which, w in ((0, 0), (1, 1), (2, 4)):
            for kt in range(KT):
                bcol[:, (i * 3 + which) * KT + kt] = biases[w][kt * 128:(kt + 1) * 128]

    outWT = np.zeros((128, KT * COUT), f32)
    for kt in range(KT):
        outWT[:, kt * COUT:(kt + 1) * COUT] = outW.T[kt * 128:(kt + 1) * 128, :]

    # conv-bias rows: (W_d @ in_b) masked by halo at use time; then cc_b, out_b
    for d in range(3):
        brow[0, L * 3 * 512 + d * 512: L * 3 * 512 + (d + 1) * 512] = \
            ccW[:, :, d] @ in_b
    brow[0, L * 3 * 512 + 3 * 512: L * 3 * 512 + 4 * 512] = cc_b
    brow[0, L * 3 * 512 + 4 * 512: L * 3 * 512 + 4 * 512 + COUT] = out_b

    tril = np.tril(np.ones((128, 128), f32)).T  # keep t<=s in [t,s] layout

    return {
        "cvWT": cvWT.astype(BF),
        "wpack": wpack.astype(BF),
        "outWT": outWT.astype(BF),
        "brow": brow.astype(BF),
        "bcol": bcol,
        "tril": tril.astype(BF),
        "ones_row": np.ones((1, 512), f32).astype(BF),
        "ones_col_bf": np.ones((128, 1), f32).astype(BF),
    }


def _prep_core_inputs(shared, inputs, b, h):
    f32 = np.float32
    x = np.asarray(inputs["x"], f32)
    s0 = h * SH
    x_sl = np.zeros((CIN, SH + 2), f32)
    lo = max(0, s0 - 2)
    x_sl[:, 2 - (s0 - lo):] = x[b, :, lo:s0 + SH]
    halo = np.ones((1, SH + 2), f32)
    if h == 0:
        halo[0, :2] = 0.0
    mcol = np.zeros((128, 2), f32)
    mcol[:, 0] = float(h)
    mcol[:, 1] = 1.0 - float(h)
    m = dict(shared)
    m["x_sl"] = x_sl.astype(BF)
    m["halo"] = halo.astype(BF)
    m["mcol"] = mcol
    return m


def _run(inputs, loop_input=False, rept=1, **kw):
    nc = _get_program(with_bias=_needs_bias(inputs), loop_input=loop_input)
    shared = _prep_shared(inputs)
    in_maps = []
    for core in range(NCORES):
        b, h = core // 2, core % 2
        m = _prep_core_inputs(shared, inputs, b, h)
        if loop_input:
            m["rept"] = np.full((1, 1), rept, np.int32)
        in_maps.append(m)
    return run_bass_kernel_spmd(nc, in_maps, core_ids=list(range(NCORES)), **kw)


def kernel(**inputs):
    res = _run(inputs)
    out = np.zeros((B, COUT, S), np.float32)
    for core in range(NCORES):
        b, h = core // 2, core % 2
        out[b, :, h * SH:(h + 1) * SH] = res.results[core]["out"]
    return out


def bench(inputs, trace_cores=(0, 1), tmpdir=None):
    """Run with NTFF tracing; returns BassKernelResults with exec_time_ns."""
    return _run(inputs, trace=True, trace_cores=list(trace_cores), tmpdir=tmpdir)
